# revision 14
# baseline (speedup 1.0000x reference)
"""Trainium2 Bass kernel for nn_Boundary_Enchance (dense_cnn), v3.

Pure data parallel: core i of 8 processes batch image i.  Heavy compute runs
in fp8 (e4m3) DoubleRow matmuls at 0.5 PE-cycles/row; weights are scaled x8
on the host so they stay in e4m3's normal range, and the x8 is folded into
downstream coefficients or the sigmoid scale.

Per-core pipeline:
  phase 1 (interleaved, PAIR-batched): per 8-row tile t (stride 6 rows)
    fuse_box' = relu(8*(1x1conv(y)) + 8*b) as ONE fp8 DoubleRow matmul
    (k-tiles = rows 0-3 / 4-7, K=20 each); two tiles share a 2-bank PSUM
    tile and ONE evacuation op (relu+bias+accum row sums, Act/DVE
    alternating) writes both F-halves of the [x | F] slots of the XF
    mega-tile (fp8).  One pair behind, the 3x3 conv front for strips
    (2c, 2c+1) runs as 6 fp8 DoubleRow matmuls (3 dx taps per strip;
    k-tile0 = x window, k-tile1 = F window, SAME padding via
    partial-column psum accumulation) into a 2-bank PSUM tile, evacuated
    by ONE relu+bias op into the FC mega-tile (fp8).
    Edge handling: uniform bias everywhere; tile0-row0 fixed by a memset,
    tile85's phantom bias rows are subtracted from the row-sum column.
  SE chain: row sums -> selection matmul -> gap -> MLP -> sigmoid -> se;
    4 DVE scalar_tensor_tensor ops build group-position LBM lhsT blocks.
  phase 2 (one-group lag): per 4-strip group q: four M=128 accumulating
    LBM matmuls put x8 logits in function-major 32-partition blocks of one
    PSUM bank (u | v | v' | ones); ONE sigmoid (scale=1/8) -> sg; DVE
    copies u to a twin tile at base 64 and ONE DVE min computes
    min(u, v') for all 4 strips (min(u+v,1) = v + min(u, 1-v));
    three M=128 8-row LC matmuls expand scalefactor -> 16 channels
    (bias via the sigma(160/8)=1 ones row) into 2-bank PSUM pair tiles,
    ONE evacuation per out-tile pair -> bf16 og ring; DMA out per 4 tiles.
"""

import numpy as np
import ml_dtypes

F8 = ml_dtypes.float8_e4m3
BF16 = ml_dtypes.bfloat16

H = 512
W = 512
SB = 6                      # conv strip rows
NT = (H + SB - 1) // SB     # 86 strips / fuse tiles
NPAIR = NT // 2             # 43 tile pairs
NO = 64                     # 8-row output tiles
NG = (NT + 3) // 4          # 22 tail groups (last has 2 strips)
NPIX = float(H * W)

# fp8 const block column offsets (wq8)
LFJ_C = 0          # fuse lhsT [20, 2, 128] -> 256 cols
WXD_C = 256        # conv lhsT 3 x [128, 2, 96] -> 3 * 192
WQ8_W = 256 + 3 * 192

# bf16 const block column offsets (wcb)
SEL_C = 0
W1L_C = 16
W2R_C = 32
LC_C = 160         # 3 x 128
PSB_C = LC_C + 384  # 4 x 128
LM_C = PSB_C + 512  # 4 x 128
WCB_W = LM_C + 512

_cache = {}


# ----------------------------------------------------------------------------
# host-side weight layout builders
# ----------------------------------------------------------------------------

def _fuse_lhsT(fuse_w):
    """[20, 256]: k-tile k, col r*16+oc <- 8*fuse_w[oc, c] at row r4*5+c
    (r = 4k + r4)."""
    out = np.zeros((20, 2, 128), np.float32)
    w8 = 8.0 * fuse_w[:, :, 0, 0]                # [16, 5]
    for r in range(8):
        k, r4 = divmod(r, 4)
        out[r4 * 5:r4 * 5 + 5, k, r * 16:r * 16 + 16] = w8.T
    return out.reshape(20, 256)


def _conv_lhsT(fc_w):
    """3 x [128, 192]: dx tap d: k-tile 0 = 8*fc_w[:, :16] row-Toeplitz over
    x rows, k-tile 1 = fc_w[:, 16:] over F' rows (F' = 8F)."""
    out = np.zeros((3, 128, 2, 96), np.float32)
    for dx in range(3):
        for i in range(SB):
            for ky in range(3):
                r = i + ky
                out[dx, r * 16:r * 16 + 16, 0, i * 16:i * 16 + 16] = \
                    8.0 * fc_w[:, 0:16, ky, dx].T
                out[dx, r * 16:r * 16 + 16, 1, i * 16:i * 16 + 16] = \
                    fc_w[:, 16:32, ky, dx].T
    return out.reshape(3, 128, 192)


def _lbm_static(fm_w, fm_b, bd_w, bd_b):
    """PSB_g / LM_g [4][128, 128] (bf16 inputs to the on-device stt):
    logits carry an extra x8; the sigmoid un-scales with scale=1/8.
    col layout: 8g+i = u (mask), 32+8g+i = v (boundary), 64+8g+i = v',
    96 = ones row -> 160."""
    fm_d = fm_w[1, :, 0, 0] - fm_w[0, :, 0, 0]
    bd_d = 8.0 * (bd_w[1, :, 0, 0] - bd_w[0, :, 0, 0])
    fm_bd = 8.0 * (fm_b[1] - fm_b[0])
    bd_bd = 8.0 * (bd_b[1] - bd_b[0])
    psb = np.zeros((4, 128, 128), np.float32)
    lm = np.zeros((4, 128, 128), np.float32)
    for g in range(4):
        for i in range(SB):
            for oc in range(16):
                lm[g, i * 16 + oc, 8 * g + i] = fm_d[oc]
            lm[g, 126, 8 * g + i] = fm_bd
            for c in range(5):
                psb[g, 96 + i * 5 + c, 32 + 8 * g + i] = bd_d[c]
                psb[g, 96 + i * 5 + c, 64 + 8 * g + i] = -bd_d[c]
            lm[g, 126, 32 + 8 * g + i] = bd_bd
            lm[g, 126, 64 + 8 * g + i] = -bd_bd
    lm[0, 126, 96] = 160.0
    return psb, lm


def _lc_lhsT(cv_w, cv_b):
    """3 x [128, 128]: phase p covers group rows r = 8p..8p+7; strip-in-group
    g = r//6, i = r%6: sf = v + min(u, v')."""
    w = cv_w[:, 0, 0, 0]
    b = cv_b
    out = np.zeros((3, 128, 128), np.float32)
    for p in range(3):
        for r8 in range(8):
            r = 8 * p + r8
            g, i = divmod(r, SB)
            for oc in range(16):
                m = r8 * 16 + oc
                out[p, 32 + 8 * g + i, m] = w[oc]
                out[p, 64 + 8 * g + i, m] = w[oc]
                out[p, 96, m] = b[oc]
    return out


def _sel_lhsT():
    """[128, 16]: sum valid rows 1..6 of each tile / (8 * NPIX)."""
    out = np.zeros((128, 16), np.float32)
    for r in range(1, 7):
        for fc in range(16):
            out[r * 16 + fc, fc] = 1.0 / (8.0 * NPIX)
    return out


def _w1_lhsT(se_w1):
    out = np.zeros((128, 16), np.float32)
    out[:16, :16] = se_w1.T
    return out


def _w2_lhsT(se_w2):
    """[128, 128]: se logits at out partitions 96 + i*5 + c."""
    out = np.zeros((128, 128), np.float32)
    for i in range(SB):
        for c in range(5):
            out[:16, 96 + i * 5 + c] = se_w2[c]
    return out


def _pack_wq8(fuse_w, fc_w):
    out = np.zeros((128, WQ8_W), np.float32)
    out[0:20, LFJ_C:LFJ_C + 256] = _fuse_lhsT(fuse_w)
    wxd = _conv_lhsT(fc_w)
    for d in range(3):
        out[:, WXD_C + 192 * d:WXD_C + 192 * (d + 1)] = wxd[d]
    return out.astype(F8)


def _pack_wcb(se_w1, se_w2, fm_w, fm_b, bd_w, bd_b, cv_w, cv_b):
    out = np.zeros((128, WCB_W), np.float32)
    out[:, SEL_C:SEL_C + 16] = _sel_lhsT()
    out[:, W1L_C:W1L_C + 16] = _w1_lhsT(se_w1)
    out[:, W2R_C:W2R_C + 128] = _w2_lhsT(se_w2)
    lc = _lc_lhsT(cv_w, cv_b)
    for p in range(3):
        out[:, LC_C + 128 * p:LC_C + 128 * (p + 1)] = lc[p]
    psb, lm = _lbm_static(fm_w, fm_b, bd_w, bd_b)
    for g in range(4):
        out[:, PSB_C + 128 * g:PSB_C + 128 * (g + 1)] = psb[g]
        out[:, LM_C + 128 * g:LM_C + 128 * (g + 1)] = lm[g]
    return out.astype(BF16)


def _pack_cbl(fuse_b, fc_b):
    """[128, 4] f32 bias columns: 0 = 8*fuse_b (per row-group), 1 = 8*fc_b
    (conv), 2 = tile-85 phantom row-sum correction, 3 spare."""
    out = np.zeros((128, 4), np.float32)
    for r in range(8):
        out[r * 16:r * 16 + 16, 0] = 8.0 * fuse_b
    for i in range(SB):
        out[i * 16:i * 16 + 16, 1] = 8.0 * fc_b
    relu8b = np.maximum(8.0 * fuse_b, 0.0).astype(F8).astype(np.float32)
    for r in range(3, 7):
        out[r * 16:r * 16 + 16, 2] = float(W) * relu8b
    return out


# ----------------------------------------------------------------------------
# bass graph
# ----------------------------------------------------------------------------

def _build():
    import concourse.bass as bass
    import concourse.bacc as bacc
    import concourse.tile as tile
    from concourse import mybir

    f32 = mybir.dt.float32
    bf16 = mybir.dt.bfloat16
    f8 = mybir.dt.float8e4
    AF = mybir.ActivationFunctionType
    ALU = mybir.AluOpType
    DR = mybir.MatmulPerfMode.DoubleRow

    nc = bacc.Bacc("TRN2", target_bir_lowering=False)
    xp_ext = nc.declare_dram_parameter("xp", [NT, 128, W], f8, isOutput=False)
    yh_ext = nc.declare_dram_parameter("yh", [20, 2 * NT, W], f8,
                                       isOutput=False)
    yo_ext = nc.declare_dram_parameter("yo", [NT, 32, W], f8, isOutput=False)
    wq8_ext = nc.declare_dram_parameter("wq8", [128, WQ8_W], f8,
                                        isOutput=False)
    wcb_ext = nc.declare_dram_parameter("wcb", [128, WCB_W], bf16,
                                        isOutput=False)
    cbl_ext = nc.declare_dram_parameter("cbl", [128, 4], f32, isOutput=False)
    out_ext = nc.declare_dram_parameter("out", [NO, 128, W], bf16,
                                        isOutput=True)

    NYH = 32   # yh ring slots (1024 cols each)
    NSG = 3
    NOGP = 4   # og ring pair slots (1024 cols each)

    with tile.TileContext(nc) as tc:
        with (
            tc.tile_pool(name="singles", bufs=1) as singles,
            tc.tile_pool(name="pa", bufs=2, space="PSUM") as pa,
            tc.tile_pool(name="pb", bufs=2, space="PSUM") as pb,
        ):
            wq8 = singles.tile([128, WQ8_W], f8, tag="wq8")
            nc.sync.dma_start(out=wq8[:, :], in_=wq8_ext[:, :])
            wcb = singles.tile([128, WCB_W], bf16, tag="wcb")
            nc.sync.dma_start(out=wcb[:, :], in_=wcb_ext[:, :])
            cbl = singles.tile([128, 4], f32, tag="cbl")
            nc.sync.dma_start(out=cbl[:, :], in_=cbl_ext[:, :])

            LFJ = wq8[0:20, LFJ_C:LFJ_C + 256].rearrange(
                "p (two m) -> p two m", two=2)
            WXD = [wq8[:, WXD_C + 192 * d:WXD_C + 192 * (d + 1)].rearrange(
                "p (two m) -> p two m", two=2) for d in range(3)]
            SEL = wcb[:, SEL_C:SEL_C + 16]
            W1L = wcb[:, W1L_C:W1L_C + 16]
            W2R = wcb[:, W2R_C:W2R_C + 128]
            LC = [wcb[:, LC_C + 128 * p:LC_C + 128 * (p + 1)]
                  for p in range(3)]
            PSB = [wcb[:, PSB_C + 128 * g:PSB_C + 128 * (g + 1)]
                   for g in range(4)]
            LM = [wcb[:, LM_C + 128 * g:LM_C + 128 * (g + 1)]
                  for g in range(4)]

            XF = singles.tile([128, NT * 1024 + 512], f8, tag="XF", name="XF")
            FC = singles.tile([128, NT * W], f8, tag="FC", name="FC")
            YH = singles.tile([20, NYH * 1024], f8, tag="YH", name="YH")
            OG = singles.tile([128, NOGP * 1024], bf16, tag="OG", name="OG")
            SG = [singles.tile([128, 1024], bf16, tag=f"SG{k}", name=f"SG{k}")
                  for k in range(NSG)]
            SGU = [singles.tile([128, 1024], bf16, tag=f"SGU{k}", name=f"SGU{k}")
                   for k in range(NSG)]
            Ra = singles.tile([128, NPAIR], f32, tag="Ra")
            nc.vector.memset(Ra[:, :], 0.0)
            LBMG = [singles.tile([128, 128], f8, tag=f"LBM{g}",
                                 name=f"LBM{g}") for g in range(4)]

            # ================= phase 1: fuse + conv fronts ==================
            def issue_in_dma(j):
                t0 = 8 * j
                if t0 >= NT:
                    return
                n = min(8, NT - t0)
                s0 = t0 % NYH
                nc.sync.dma_start(
                    out=YH[0:20, s0 * 1024:(s0 + n) * 1024].rearrange(
                        "p (s j) -> p s j", s=2 * n),
                    in_=yh_ext[:, 2 * t0:2 * (t0 + n), :])
                nc.sync.dma_start(
                    out=XF[:, t0 * 1024:(t0 + n) * 1024].rearrange(
                        "p (s j) -> p s j", s=n)[:, :, 0:W],
                    in_=xp_ext[t0:t0 + n, :, :].rearrange("s p j -> p s j"))
                nc.sync.dma_start(
                    out=FC[96:128, t0 * W:(t0 + n) * W].rearrange(
                        "p (s j) -> p s j", s=n),
                    in_=yo_ext[t0:t0 + n, :, :].rearrange("s p j -> p s j"))

            def issue_fuse_pair(k):
                t0 = 2 * k
                if t0 % 8 == 0:
                    issue_in_dma(t0 // 8 + 3)
                fps = pa.tile([128, 1024], f32, tag="a")
                for h in range(2):
                    t = t0 + h
                    s = t % NYH
                    rhs = YH[0:20, s * 1024:(s + 1) * 1024].rearrange(
                        "p (two j) -> p two j", two=2)
                    nc.tensor.matmul(fps[:, h * W:(h + 1) * W], lhsT=LFJ,
                                     rhs=rhs, start=True, stop=True,
                                     perf_mode=DR)
                # one evac for both halves -> F-halves of XF slots t0, t0+1
                dst = XF[:, t0 * 1024 + W:t0 * 1024 + W + 2048].rearrange(
                    "p (s j) -> p s j", s=2)[:, :, 0:W]
                nc.vector.tensor_scalar(out=dst, in0=fps[:, :],
                                        scalar1=cbl[:, 0:1],
                                        scalar2=0.0,
                                        op0=ALU.add, op1=ALU.max,
                                        accum_out=Ra[:, k:k + 1])
                if k == 0:
                    # tile0 row0 is image row -1: kill its bias-only relu
                    nc.vector.memset(XF[0:16, W:2 * W], 0.0)

            def issue_front_pair(c):
                cps = pb.tile([96, 1024], f32, tag="b")
                for h in range(2):
                    s = 2 * c + h
                    v = XF[:, s * 1024:(s + 1) * 1024].rearrange(
                        "p (two j) -> p two j", two=2)
                    o = h * W
                    nc.tensor.matmul(cps[:, o:o + W], lhsT=WXD[1],
                                     rhs=v[:, :, 0:W],
                                     start=True, stop=False, perf_mode=DR)
                    nc.tensor.matmul(cps[:, o + 1:o + W], lhsT=WXD[0],
                                     rhs=v[:, :, 0:W - 1],
                                     start=False, stop=False, perf_mode=DR)
                    nc.tensor.matmul(cps[:, o:o + W - 1], lhsT=WXD[2],
                                     rhs=v[:, :, 1:W],
                                     start=False, stop=True, perf_mode=DR)
                dst = FC[0:96, 2 * c * W:(2 * c + 2) * W]
                nc.scalar.activation(out=dst, in_=cps[:, :], func=AF.Relu,
                                     bias=cbl[0:96, 1:2])

            # ================= SE chain =====================================
            def issue_se():
                # tile-85 phantom bias rows leaked into Ra col 42: subtract
                nc.vector.tensor_scalar(out=Ra[:, NPAIR - 1:NPAIR],
                                        in0=Ra[:, NPAIR - 1:NPAIR],
                                        scalar1=cbl[:, 2:3], scalar2=0.0,
                                        op0=ALU.subtract, op1=ALU.add)
                Rbf = singles.tile([128, NPAIR], bf16, tag="Rbf")
                nc.vector.tensor_copy(out=Rbf[:, :], in_=Ra[:, :])
                gps = pb.tile([16, NPAIR], f32, tag="b")
                nc.tensor.matmul(gps[:, :], lhsT=SEL, rhs=Rbf[:, :],
                                 start=True, stop=True)
                gap_f = singles.tile([16, 1], f32, tag="gapf")
                nc.vector.reduce_sum(out=gap_f[:, :], in_=gps[:, :],
                                     axis=mybir.AxisListType.X)
                gap_bf = singles.tile([128, 1], bf16, tag="gap")
                nc.vector.memset(gap_bf[:, :], 0.0)
                nc.vector.tensor_copy(out=gap_bf[0:16, :], in_=gap_f[:, :])
                hps = pb.tile([16, 1], f32, tag="b")
                nc.tensor.matmul(hps[:, :], lhsT=W1L, rhs=gap_bf[:, :],
                                 start=True, stop=True)
                h_bf = singles.tile([128, 1], bf16, tag="hbf")
                nc.vector.memset(h_bf[:, :], 0.0)
                nc.scalar.activation(out=h_bf[0:16, :], in_=hps[:, :],
                                     func=AF.Relu)
                sps = pb.tile([128, 1], f32, tag="b")
                nc.tensor.matmul(sps[:, :], lhsT=W2R, rhs=h_bf[:, :],
                                 start=True, stop=True)
                se_bc = singles.tile([128, 1], f32, tag="sebc")
                nc.scalar.activation(out=se_bc[:, :], in_=sps[:, :],
                                     func=AF.Sigmoid)
                for g in range(4):
                    nc.vector.scalar_tensor_tensor(
                        out=LBMG[g][:, :], in0=PSB[g], scalar=se_bc[:, :],
                        in1=LM[g], op0=ALU.mult, op1=ALU.add)

            # ================= phase 2: tails ===============================
            cvt = {}

            def issue_head_pair(j):
                mb = pb.tile([128, 1024], f32, tag="b", name=f"mb{j}")
                for h in range(2):
                    q = 2 * j + h
                    ns = min(4, NT - 4 * q)
                    for g in range(ns):
                        u = 4 * q + g
                        nc.tensor.matmul(mb[:, h * W:(h + 1) * W],
                                         lhsT=LBMG[g][:, :],
                                         rhs=FC[:, u * W:(u + 1) * W],
                                         start=(g == 0), stop=(g == ns - 1))
                sg = SG[j % NSG]
                sgu = SGU[j % NSG]
                nc.scalar.activation(out=sg[:, :], in_=mb[:, :],
                                     func=AF.Sigmoid, scale=0.125)
                nc.vector.tensor_copy(out=sgu[64:96, :], in_=sg[0:32, :])
                nc.vector.tensor_tensor(out=sg[64:96, :], in0=sgu[64:96, :],
                                        in1=sg[64:96, :], op=ALU.min)

            def issue_head_pair_split(j):
                mb = pb.tile([128, 1024], f32, tag="b", name=f"mb{j}")
                sg = SG[j % NSG]
                sgu = SGU[j % NSG]
                for h in range(2):
                    q = 2 * j + h
                    ns = min(4, NT - 4 * q)
                    for g in range(ns):
                        u = 4 * q + g
                        nc.tensor.matmul(mb[:, h * W:(h + 1) * W],
                                         lhsT=LBMG[g][:, :],
                                         rhs=FC[:, u * W:(u + 1) * W],
                                         start=(g == 0), stop=(g == ns - 1))
                    hv = slice(h * W, (h + 1) * W)
                    nc.scalar.activation(out=sg[:, hv], in_=mb[:, hv],
                                         func=AF.Sigmoid, scale=0.125)
                nc.vector.tensor_copy(out=sgu[64:96, :], in_=sg[0:32, :])
                nc.vector.tensor_tensor(out=sg[64:96, :], in0=sgu[64:96, :],
                                        in1=sg[64:96, :], op=ALU.min)

            def issue_lc(q):
                sg = SG[(q // 2) % NSG][:, (q % 2) * W:(q % 2 + 1) * W]
                np_ = 3 if q < NG - 1 else 1
                for p in range(np_):
                    tau = 3 * q + p
                    pi, h = divmod(tau, 2)
                    if h == 0:
                        cvt[pi] = pa.tile([128, 1024], f32, tag="a",
                                          name=f"cv{pi}")
                    ops = cvt[pi]
                    nc.tensor.matmul(ops[:, h * W:(h + 1) * W], lhsT=LC[p],
                                     rhs=sg, start=True, stop=True)
                    if h == 1:
                        dst = OG[:, (pi % NOGP) * 1024:
                                 (pi % NOGP + 1) * 1024]
                        if pi % 2 == 0:
                            nc.scalar.activation(out=dst, in_=ops[:, :],
                                                 func=AF.Copy)
                        else:
                            nc.vector.tensor_copy(out=dst, in_=ops[:, :])
                        del cvt[pi]
                        if pi % 2 == 1:
                            t0 = 2 * (pi - 1)
                            c0 = ((pi - 1) % NOGP) * 1024
                            nc.sync.dma_start(
                                out=out_ext[t0:t0 + 4, :, :].rearrange(
                                    "s p j -> p s j"),
                                in_=OG[:, c0:c0 + 4 * W].rearrange(
                                    "p (s j) -> p s j", s=4))

            for j in range(3):
                issue_in_dma(j)
            for k in range(NPAIR + 3):
                if k < NPAIR:
                    issue_fuse_pair(k)
                if k >= 3:
                    issue_front_pair(k - 3)
            issue_se()
            NJ = (NG + 1) // 2
            for j in range(NJ + 1):
                if j < NJ:
                    issue_head_pair(j)
                if j >= 1:
                    issue_lc(2 * (j - 1))
                    issue_lc(2 * (j - 1) + 1)
    nc.compile()
    return nc


# ----------------------------------------------------------------------------
# entry point
# ----------------------------------------------------------------------------

LAST_RESULT = None


def prepare(x, y, fuse_w, fuse_b, se_w1, se_w2, bd_w, bd_b,
            fc_w, fc_b, fm_w, fm_b, cv_w, cv_b):
    if "nc" not in _cache:
        _cache["nc"] = _build()
    nc = _cache["nc"]

    g = {}
    for k, v in (("fuse_w", fuse_w), ("fuse_b", fuse_b), ("se_w1", se_w1),
                 ("se_w2", se_w2), ("bd_w", bd_w), ("bd_b", bd_b),
                 ("fc_w", fc_w), ("fc_b", fc_b), ("fm_w", fm_w),
                 ("fm_b", fm_b), ("cv_w", cv_w), ("cv_b", cv_b)):
        g[k] = np.asarray(v, np.float32)

    wq8 = _pack_wq8(g["fuse_w"], g["fc_w"])
    wcb = _pack_wcb(g["se_w1"], g["se_w2"], g["fm_w"], g["fm_b"],
                    g["bd_w"], g["bd_b"], g["cv_w"], g["cv_b"])
    cbl = _pack_cbl(g["fuse_b"], g["fc_b"])

    x8 = np.asarray(x, np.float32).astype(F8)
    y8 = np.asarray(y, np.float32).astype(F8)
    B = x8.shape[0]

    # x: 8-row overlapping windows, stride 6, partition r*16+ic
    xpad = np.zeros((B, 16, 6 * NT + 8, W), F8)
    xpad[:, :, 1:H + 1, :] = x8
    ridx = 6 * np.arange(NT)[:, None] + np.arange(8)[None, :]
    xp = xpad[:, :, ridx, :].transpose(0, 2, 3, 1, 4).reshape(B, NT, 128, W)

    # yh: [20, 2*NT, W]: partition r4*5+c, col-block 2t+k = ypad row 6t+4k+r4
    ypad = np.zeros((B, 5, 6 * NT + 8, W), F8)
    ypad[:, :, 1:H + 1, :] = y8
    yh = ypad[:, :, ridx, :]                     # [B, 5, NT, 8, W]
    yh = yh.reshape(B, 5, NT, 2, 4, W).transpose(0, 4, 1, 2, 3, 5) \
           .reshape(B, 20, 2 * NT, W)

    # yo: [NT, 32, W]: rows i*5+c = y row 6s+i; row 30 ones; row 31 zero
    yo = np.zeros((B, NT, 32, W), F8)
    cidx = 6 * np.arange(NT)[:, None] + 1 + np.arange(6)[None, :]
    yv = ypad[:, :, cidx, :]                     # [B, 5, NT, 6, W]
    yo[:, :, 0:30, :] = yv.transpose(0, 2, 3, 1, 4).reshape(B, NT, 30, W)
    yo[:, :, 30, :] = 1.0

    in_maps = [
        {"xp": np.ascontiguousarray(xp[i]),
         "yh": np.ascontiguousarray(yh[i]),
         "yo": np.ascontiguousarray(yo[i]),
         "wq8": wq8, "wcb": wcb, "cbl": cbl}
        for i in range(B)
    ]
    return nc, in_maps


def kernel(x, y, fuse_w, fuse_b, se_w1, se_w2, bd_w, bd_b,
           fc_w, fc_b, fm_w, fm_b, cv_w, cv_b):
    global LAST_RESULT
    from concourse.bass_utils import run_bass_kernel_spmd

    nc, in_maps = prepare(x, y, fuse_w, fuse_b, se_w1, se_w2, bd_w, bd_b,
                          fc_w, fc_b, fm_w, fm_b, cv_w, cv_b)
    res = run_bass_kernel_spmd(nc, in_maps, core_ids=list(range(8)))
    LAST_RESULT = res
    outs = []
    for i in range(8):
        ot = np.asarray(res.results[i]["out"], np.float32)  # [NO, 128, W]
        full = ot.reshape(NO, 8, 16, W).transpose(2, 0, 1, 3) \
                 .reshape(16, NO * 8, W)
        outs.append(full)
    return np.stack(outs)


# revision 22
# speedup vs baseline: 1.0082x; 1.0082x over previous
"""Trainium2 Bass kernel for nn_Boundary_Enchance (dense_cnn), v3.

Pure data parallel: core i of 8 processes batch image i.  Heavy compute runs
in fp8 (e4m3) DoubleRow matmuls at 0.5 PE-cycles/row; weights are scaled x8
on the host so they stay in e4m3's normal range, and the x8 is folded into
downstream coefficients or the sigmoid scale.

Per-core pipeline:
  phase 1 (interleaved, PAIR-batched): per 8-row tile t (stride 6 rows)
    fuse_box' = relu(8*(1x1conv(y)) + 8*b) as ONE fp8 DoubleRow matmul
    (k-tiles = rows 0-3 / 4-7, K=20 each); two tiles share a 2-bank PSUM
    tile and ONE evacuation op (relu+bias+accum row sums, Act/DVE
    alternating) writes both F-halves of the [x | F] slots of the XF
    mega-tile (fp8).  One pair behind, the 3x3 conv front for strips
    (2c, 2c+1) runs as 6 fp8 DoubleRow matmuls (3 dx taps per strip;
    k-tile0 = x window, k-tile1 = F window, SAME padding via
    partial-column psum accumulation) into a 2-bank PSUM tile, evacuated
    by ONE relu+bias op into the FC mega-tile (fp8).
    Edge handling: uniform bias everywhere; tile0-row0 fixed by a memset,
    tile85's phantom bias rows are subtracted from the row-sum column.
  SE chain: row sums -> selection matmul -> gap -> MLP -> sigmoid -> se;
    4 DVE scalar_tensor_tensor ops build group-position LBM lhsT blocks.
  phase 2 (one-group lag): per 4-strip group q: four M=128 accumulating
    LBM matmuls put x8 logits in function-major 32-partition blocks of one
    PSUM bank (u | v | v' | ones); ONE sigmoid (scale=1/8) -> sg; DVE
    copies u to a twin tile at base 64 and ONE DVE min computes
    min(u, v') for all 4 strips (min(u+v,1) = v + min(u, 1-v));
    three M=128 8-row LC matmuls expand scalefactor -> 16 channels
    (bias via the sigma(160/8)=1 ones row) into 2-bank PSUM pair tiles,
    ONE evacuation per out-tile pair -> bf16 og ring; DMA out per 4 tiles.
"""

import numpy as np
import ml_dtypes

F8 = ml_dtypes.float8_e4m3
BF16 = ml_dtypes.bfloat16

H = 512
W = 512
SB = 6                      # conv strip rows
NT = (H + SB - 1) // SB     # 86 strips / fuse tiles
NPAIR = NT // 2             # 43 tile pairs
NO = 64                     # 8-row output tiles
NG = (NT + 3) // 4          # 22 tail groups (last has 2 strips)
NPIX = float(H * W)

# fp8 const block column offsets (wq8)
LFJ_C = 0          # fuse lhsT [20, 2, 128] -> 256 cols
WXD_C = 256        # conv lhsT 3 x [128, 2, 96] -> 3 * 192
WQ8_W = 256 + 3 * 192

# bf16 const block column offsets (wcb)
SEL_C = 0
W1L_C = 16
W2R_C = 32
LC_C = 160         # 3 x 128
PSB_C = LC_C + 384  # 4 x 128
LM_C = PSB_C + 512  # 4 x 128
WCB_W = LM_C + 512

_cache = {}


# ----------------------------------------------------------------------------
# host-side weight layout builders
# ----------------------------------------------------------------------------

def _fuse_lhsT(fuse_w):
    """[20, 256]: k-tile k, col r*16+oc <- 8*fuse_w[oc, c] at row r4*5+c
    (r = 4k + r4)."""
    out = np.zeros((20, 2, 128), np.float32)
    w8 = 8.0 * fuse_w[:, :, 0, 0]                # [16, 5]
    for r in range(8):
        k, r4 = divmod(r, 4)
        out[r4 * 5:r4 * 5 + 5, k, r * 16:r * 16 + 16] = w8.T
    return out.reshape(20, 256)


def _conv_lhsT(fc_w):
    """3 x [128, 192]: dx tap d: k-tile 0 = 8*fc_w[:, :16] row-Toeplitz over
    x rows, k-tile 1 = fc_w[:, 16:] over F' rows (F' = 8F)."""
    out = np.zeros((3, 128, 2, 96), np.float32)
    for dx in range(3):
        for i in range(SB):
            for ky in range(3):
                r = i + ky
                out[dx, r * 16:r * 16 + 16, 0, i * 16:i * 16 + 16] = \
                    8.0 * fc_w[:, 0:16, ky, dx].T
                out[dx, r * 16:r * 16 + 16, 1, i * 16:i * 16 + 16] = \
                    fc_w[:, 16:32, ky, dx].T
    return out.reshape(3, 128, 192)


def _lbm_static(fm_w, fm_b, bd_w, bd_b):
    """PSB_g / LM_g [4][128, 128] (bf16 inputs to the on-device stt):
    logits carry an extra x8; the sigmoid un-scales with scale=1/8.
    col layout: 8g+i = u (mask), 32+8g+i = v (boundary), 64+8g+i = v',
    96 = ones row -> 160."""
    fm_d = fm_w[1, :, 0, 0] - fm_w[0, :, 0, 0]
    bd_d = 8.0 * (bd_w[1, :, 0, 0] - bd_w[0, :, 0, 0])
    fm_bd = 8.0 * (fm_b[1] - fm_b[0])
    bd_bd = 8.0 * (bd_b[1] - bd_b[0])
    psb = np.zeros((4, 128, 128), np.float32)
    lm = np.zeros((4, 128, 128), np.float32)
    for g in range(4):
        for i in range(SB):
            for oc in range(16):
                lm[g, i * 16 + oc, 8 * g + i] = fm_d[oc]
            lm[g, 126, 8 * g + i] = fm_bd
            for c in range(5):
                psb[g, 96 + i * 5 + c, 32 + 8 * g + i] = bd_d[c]
                psb[g, 96 + i * 5 + c, 64 + 8 * g + i] = -bd_d[c]
            lm[g, 126, 32 + 8 * g + i] = bd_bd
            lm[g, 126, 64 + 8 * g + i] = -bd_bd
    lm[0, 126, 96] = 160.0
    return psb, lm


def _lc_lhsT(cv_w, cv_b):
    """3 x [128, 128]: phase p covers group rows r = 8p..8p+7; strip-in-group
    g = r//6, i = r%6: sf = v + min(u, v')."""
    w = cv_w[:, 0, 0, 0]
    b = cv_b
    out = np.zeros((3, 128, 128), np.float32)
    for p in range(3):
        for r8 in range(8):
            r = 8 * p + r8
            g, i = divmod(r, SB)
            for oc in range(16):
                m = r8 * 16 + oc
                out[p, 32 + 8 * g + i, m] = w[oc]
                out[p, 64 + 8 * g + i, m] = w[oc]
                out[p, 96, m] = b[oc]
    return out


def _sel_lhsT():
    """[128, 16]: sum valid rows 1..6 of each tile / (8 * NPIX)."""
    out = np.zeros((128, 16), np.float32)
    for r in range(1, 7):
        for fc in range(16):
            out[r * 16 + fc, fc] = 1.0 / (8.0 * NPIX)
    return out


def _w1_lhsT(se_w1):
    out = np.zeros((128, 16), np.float32)
    out[:16, :16] = se_w1.T
    return out


def _w2_lhsT(se_w2):
    """[128, 128]: se logits at out partitions 96 + i*5 + c."""
    out = np.zeros((128, 128), np.float32)
    for i in range(SB):
        for c in range(5):
            out[:16, 96 + i * 5 + c] = se_w2[c]
    return out


def _pack_wq8(fuse_w, fc_w):
    out = np.zeros((128, WQ8_W), np.float32)
    out[0:20, LFJ_C:LFJ_C + 256] = _fuse_lhsT(fuse_w)
    wxd = _conv_lhsT(fc_w)
    for d in range(3):
        out[:, WXD_C + 192 * d:WXD_C + 192 * (d + 1)] = wxd[d]
    return out.astype(F8)


def _pack_wcb(se_w1, se_w2, fm_w, fm_b, bd_w, bd_b, cv_w, cv_b):
    out = np.zeros((128, WCB_W), np.float32)
    out[:, SEL_C:SEL_C + 16] = _sel_lhsT()
    out[:, W1L_C:W1L_C + 16] = _w1_lhsT(se_w1)
    out[:, W2R_C:W2R_C + 128] = _w2_lhsT(se_w2)
    lc = _lc_lhsT(cv_w, cv_b)
    for p in range(3):
        out[:, LC_C + 128 * p:LC_C + 128 * (p + 1)] = lc[p]
    psb, lm = _lbm_static(fm_w, fm_b, bd_w, bd_b)
    for g in range(4):
        out[:, PSB_C + 128 * g:PSB_C + 128 * (g + 1)] = psb[g]
        out[:, LM_C + 128 * g:LM_C + 128 * (g + 1)] = lm[g]
    return out.astype(BF16)


def _pack_cbl(fuse_b, fc_b):
    """[128, 4] f32 bias columns: 0 = 8*fuse_b (per row-group), 1 = 8*fc_b
    (conv), 2 = tile-85 phantom row-sum correction, 3 spare."""
    out = np.zeros((128, 4), np.float32)
    for r in range(8):
        out[r * 16:r * 16 + 16, 0] = 8.0 * fuse_b
    for i in range(SB):
        out[i * 16:i * 16 + 16, 1] = 8.0 * fc_b
    relu8b = np.maximum(8.0 * fuse_b, 0.0).astype(F8).astype(np.float32)
    for r in range(3, 7):
        out[r * 16:r * 16 + 16, 2] = float(W) * relu8b
    return out


# ----------------------------------------------------------------------------
# bass graph
# ----------------------------------------------------------------------------

def _build():
    import concourse.bass as bass
    import concourse.bacc as bacc
    import concourse.tile as tile
    from concourse import mybir

    f32 = mybir.dt.float32
    bf16 = mybir.dt.bfloat16
    f8 = mybir.dt.float8e4
    AF = mybir.ActivationFunctionType
    ALU = mybir.AluOpType
    DR = mybir.MatmulPerfMode.DoubleRow

    nc = bacc.Bacc("TRN2", target_bir_lowering=False)
    xp_ext = nc.declare_dram_parameter("xp", [NT, 128, W], f8, isOutput=False)
    yh_ext = nc.declare_dram_parameter("yh", [20, 2 * NT, W], f8,
                                       isOutput=False)
    yo_ext = nc.declare_dram_parameter("yo", [NT, 32, W], f8, isOutput=False)
    wq8_ext = nc.declare_dram_parameter("wq8", [128, WQ8_W], f8,
                                        isOutput=False)
    wcb_ext = nc.declare_dram_parameter("wcb", [128, WCB_W], bf16,
                                        isOutput=False)
    cbl_ext = nc.declare_dram_parameter("cbl", [128, 4], f32, isOutput=False)
    out_ext = nc.declare_dram_parameter("out", [NO, 128, W], bf16,
                                        isOutput=True)

    NYH = 32   # yh ring slots (1024 cols each)
    NSG = 3
    NOGP = 4   # og ring pair slots (1024 cols each)

    with tile.TileContext(nc) as tc:
        with (
            tc.tile_pool(name="singles", bufs=1) as singles,
            tc.tile_pool(name="pa", bufs=2, space="PSUM") as pa,
            tc.tile_pool(name="pb", bufs=2, space="PSUM") as pb,
        ):
            wq8 = singles.tile([128, WQ8_W], f8, tag="wq8")
            nc.sync.dma_start(out=wq8[:, :], in_=wq8_ext[:, :])
            wcb = singles.tile([128, WCB_W], bf16, tag="wcb")
            nc.sync.dma_start(out=wcb[:, :], in_=wcb_ext[:, :])
            cbl = singles.tile([128, 4], f32, tag="cbl")
            nc.sync.dma_start(out=cbl[:, :], in_=cbl_ext[:, :])

            LFJ = wq8[0:20, LFJ_C:LFJ_C + 256].rearrange(
                "p (two m) -> p two m", two=2)
            WXD = [wq8[:, WXD_C + 192 * d:WXD_C + 192 * (d + 1)].rearrange(
                "p (two m) -> p two m", two=2) for d in range(3)]
            SEL = wcb[:, SEL_C:SEL_C + 16]
            W1L = wcb[:, W1L_C:W1L_C + 16]
            W2R = wcb[:, W2R_C:W2R_C + 128]
            LC = [wcb[:, LC_C + 128 * p:LC_C + 128 * (p + 1)]
                  for p in range(3)]
            PSB = [wcb[:, PSB_C + 128 * g:PSB_C + 128 * (g + 1)]
                   for g in range(4)]
            LM = [wcb[:, LM_C + 128 * g:LM_C + 128 * (g + 1)]
                  for g in range(4)]

            XF = singles.tile([128, NT * 1024 + 512], f8, tag="XF", name="XF")
            FC = singles.tile([128, NT * W], f8, tag="FC", name="FC")
            YH = singles.tile([20, NYH * 1024], f8, tag="YH", name="YH")
            OG = singles.tile([128, NOGP * 1024], bf16, tag="OG", name="OG")
            SG = [singles.tile([128, 1024], bf16, tag=f"SG{k}", name=f"SG{k}")
                  for k in range(NSG)]
            SGU = [singles.tile([128, 1024], bf16, tag=f"SGU{k}", name=f"SGU{k}")
                   for k in range(NSG)]
            Ra = singles.tile([128, NPAIR], f32, tag="Ra")
            nc.vector.memset(Ra[:, :], 0.0)
            LBMG = [singles.tile([128, 128], f8, tag=f"LBM{g}",
                                 name=f"LBM{g}") for g in range(4)]

            # ================= phase 1: fuse + conv fronts ==================
            def issue_in_dma(j):
                t0 = 8 * j
                if t0 >= NT:
                    return
                n = min(8, NT - t0)
                s0 = t0 % NYH
                nc.sync.dma_start(
                    out=YH[0:20, s0 * 1024:(s0 + n) * 1024].rearrange(
                        "p (s j) -> p s j", s=2 * n),
                    in_=yh_ext[:, 2 * t0:2 * (t0 + n), :])
                nc.sync.dma_start(
                    out=XF[:, t0 * 1024:(t0 + n) * 1024].rearrange(
                        "p (s j) -> p s j", s=n)[:, :, 0:W],
                    in_=xp_ext[t0:t0 + n, :, :].rearrange("s p j -> p s j"))
                nc.sync.dma_start(
                    out=FC[96:128, t0 * W:(t0 + n) * W].rearrange(
                        "p (s j) -> p s j", s=n),
                    in_=yo_ext[t0:t0 + n, :, :].rearrange("s p j -> p s j"))

            def issue_fuse_pair(k):
                t0 = 2 * k
                if t0 % 8 == 0:
                    issue_in_dma(t0 // 8 + 3)
                fps = pa.tile([128, 1024], f32, tag="a")
                for h in range(2):
                    t = t0 + h
                    s = t % NYH
                    rhs = YH[0:20, s * 1024:(s + 1) * 1024].rearrange(
                        "p (two j) -> p two j", two=2)
                    nc.tensor.matmul(fps[:, h * W:(h + 1) * W], lhsT=LFJ,
                                     rhs=rhs, start=True, stop=True,
                                     perf_mode=DR)
                # one evac for both halves -> F-halves of XF slots t0, t0+1
                dst = XF[:, t0 * 1024 + W:t0 * 1024 + W + 2048].rearrange(
                    "p (s j) -> p s j", s=2)[:, :, 0:W]
                nc.vector.tensor_scalar(out=dst, in0=fps[:, :],
                                        scalar1=cbl[:, 0:1],
                                        scalar2=0.0,
                                        op0=ALU.add, op1=ALU.max,
                                        accum_out=Ra[:, k:k + 1])
                if k == 0:
                    # tile0 row0 is image row -1: kill its bias-only relu
                    nc.vector.memset(XF[0:16, W:2 * W], 0.0)

            def issue_front_pair(c):
                cps = pb.tile([96, 1024], f32, tag="b")
                for h in range(2):
                    s = 2 * c + h
                    v = XF[:, s * 1024:(s + 1) * 1024].rearrange(
                        "p (two j) -> p two j", two=2)
                    o = h * W
                    nc.tensor.matmul(cps[:, o:o + W], lhsT=WXD[1],
                                     rhs=v[:, :, 0:W],
                                     start=True, stop=False, perf_mode=DR)
                    nc.tensor.matmul(cps[:, o + 1:o + W], lhsT=WXD[0],
                                     rhs=v[:, :, 0:W - 1],
                                     start=False, stop=False, perf_mode=DR)
                    nc.tensor.matmul(cps[:, o:o + W - 1], lhsT=WXD[2],
                                     rhs=v[:, :, 1:W],
                                     start=False, stop=True, perf_mode=DR)
                dst = FC[0:96, 2 * c * W:(2 * c + 2) * W]
                nc.scalar.activation(out=dst, in_=cps[:, :], func=AF.Relu,
                                     bias=cbl[0:96, 1:2])

            # ================= SE chain =====================================
            def issue_se():
                # tile-85 phantom bias rows leaked into Ra col 42: subtract
                nc.vector.tensor_scalar(out=Ra[:, NPAIR - 1:NPAIR],
                                        in0=Ra[:, NPAIR - 1:NPAIR],
                                        scalar1=cbl[:, 2:3], scalar2=0.0,
                                        op0=ALU.subtract, op1=ALU.add)
                Rbf = singles.tile([128, NPAIR], bf16, tag="Rbf")
                nc.vector.tensor_copy(out=Rbf[:, :], in_=Ra[:, :])
                gps = pb.tile([16, NPAIR], f32, tag="b")
                nc.tensor.matmul(gps[:, :], lhsT=SEL, rhs=Rbf[:, :],
                                 start=True, stop=True)
                gap_f = singles.tile([16, 1], f32, tag="gapf")
                nc.vector.reduce_sum(out=gap_f[:, :], in_=gps[:, :],
                                     axis=mybir.AxisListType.X)
                gap_bf = singles.tile([128, 1], bf16, tag="gap")
                nc.vector.memset(gap_bf[:, :], 0.0)
                nc.vector.tensor_copy(out=gap_bf[0:16, :], in_=gap_f[:, :])
                hps = pb.tile([16, 1], f32, tag="b")
                nc.tensor.matmul(hps[:, :], lhsT=W1L, rhs=gap_bf[:, :],
                                 start=True, stop=True)
                h_bf = singles.tile([128, 1], bf16, tag="hbf")
                nc.vector.memset(h_bf[:, :], 0.0)
                nc.scalar.activation(out=h_bf[0:16, :], in_=hps[:, :],
                                     func=AF.Relu)
                sps = pb.tile([128, 1], f32, tag="b")
                nc.tensor.matmul(sps[:, :], lhsT=W2R, rhs=h_bf[:, :],
                                 start=True, stop=True)
                se_bc = singles.tile([128, 1], f32, tag="sebc")
                nc.scalar.activation(out=se_bc[:, :], in_=sps[:, :],
                                     func=AF.Sigmoid)
                for g in range(4):
                    nc.vector.scalar_tensor_tensor(
                        out=LBMG[g][:, :], in0=PSB[g], scalar=se_bc[:, :],
                        in1=LM[g], op0=ALU.mult, op1=ALU.add)

            # ================= phase 2: tails ===============================
            cvt = {}
            p2ctr = [0]

            def p2tile(name):
                i = p2ctr[0]
                p2ctr[0] += 1
                pool = pa if i % 2 == 0 else pb
                return pool.tile([128, 1024], f32,
                                 tag=("a" if i % 2 == 0 else "b"), name=name)

            def issue_head_pair(j):
                mb = p2tile(f"mb{j}")
                for h in range(2):
                    q = 2 * j + h
                    ns = min(4, NT - 4 * q)
                    for g in range(ns):
                        u = 4 * q + g
                        nc.tensor.matmul(mb[:, h * W:(h + 1) * W],
                                         lhsT=LBMG[g][:, :],
                                         rhs=FC[:, u * W:(u + 1) * W],
                                         start=(g == 0), stop=(g == ns - 1))
                sg = SG[j % NSG]
                sgu = SGU[j % NSG]
                nc.scalar.activation(out=sg[:, :], in_=mb[:, :],
                                     func=AF.Sigmoid, scale=0.125)
                nc.vector.tensor_copy(out=sgu[64:96, :], in_=sg[0:32, :])
                nc.vector.tensor_tensor(out=sg[64:96, :], in0=sgu[64:96, :],
                                        in1=sg[64:96, :], op=ALU.min)

            def issue_head_pair_split(j):
                mb = pb.tile([128, 1024], f32, tag="b", name=f"mb{j}")
                sg = SG[j % NSG]
                sgu = SGU[j % NSG]
                for h in range(2):
                    q = 2 * j + h
                    ns = min(4, NT - 4 * q)
                    for g in range(ns):
                        u = 4 * q + g
                        nc.tensor.matmul(mb[:, h * W:(h + 1) * W],
                                         lhsT=LBMG[g][:, :],
                                         rhs=FC[:, u * W:(u + 1) * W],
                                         start=(g == 0), stop=(g == ns - 1))
                    hv = slice(h * W, (h + 1) * W)
                    nc.scalar.activation(out=sg[:, hv], in_=mb[:, hv],
                                         func=AF.Sigmoid, scale=0.125)
                nc.vector.tensor_copy(out=sgu[64:96, :], in_=sg[0:32, :])
                nc.vector.tensor_tensor(out=sg[64:96, :], in0=sgu[64:96, :],
                                        in1=sg[64:96, :], op=ALU.min)

            def issue_lc(q):
                sg = SG[(q // 2) % NSG][:, (q % 2) * W:(q % 2 + 1) * W]
                np_ = 3 if q < NG - 1 else 1
                for p in range(np_):
                    tau = 3 * q + p
                    pi, h = divmod(tau, 2)
                    if h == 0:
                        cvt[pi] = p2tile(f"cv{pi}")
                    ops = cvt[pi]
                    nc.tensor.matmul(ops[:, h * W:(h + 1) * W], lhsT=LC[p],
                                     rhs=sg, start=True, stop=True)
                    if h == 1:
                        dst = OG[:, (pi % NOGP) * 1024:
                                 (pi % NOGP + 1) * 1024]
                        if pi % 2 == 0:
                            nc.scalar.activation(out=dst, in_=ops[:, :],
                                                 func=AF.Copy)
                        else:
                            nc.vector.tensor_copy(out=dst, in_=ops[:, :])
                        del cvt[pi]
                        if pi % 2 == 1:
                            t0 = 2 * (pi - 1)
                            c0 = ((pi - 1) % NOGP) * 1024
                            nc.sync.dma_start(
                                out=out_ext[t0:t0 + 4, :, :].rearrange(
                                    "s p j -> p s j"),
                                in_=OG[:, c0:c0 + 4 * W].rearrange(
                                    "p (s j) -> p s j", s=4))

            for j in range(3):
                issue_in_dma(j)
            for k in range(NPAIR + 2):
                if k < NPAIR:
                    issue_fuse_pair(k)
                if k >= 2:
                    issue_front_pair(k - 2)
            issue_se()
            NJ = (NG + 1) // 2
            for j in range(NJ + 1):
                if j < NJ:
                    issue_head_pair(j)
                if j >= 1:
                    issue_lc(2 * (j - 1))
                    issue_lc(2 * (j - 1) + 1)
    nc.compile()
    return nc


# ----------------------------------------------------------------------------
# entry point
# ----------------------------------------------------------------------------

LAST_RESULT = None


def prepare(x, y, fuse_w, fuse_b, se_w1, se_w2, bd_w, bd_b,
            fc_w, fc_b, fm_w, fm_b, cv_w, cv_b):
    if "nc" not in _cache:
        _cache["nc"] = _build()
    nc = _cache["nc"]

    g = {}
    for k, v in (("fuse_w", fuse_w), ("fuse_b", fuse_b), ("se_w1", se_w1),
                 ("se_w2", se_w2), ("bd_w", bd_w), ("bd_b", bd_b),
                 ("fc_w", fc_w), ("fc_b", fc_b), ("fm_w", fm_w),
                 ("fm_b", fm_b), ("cv_w", cv_w), ("cv_b", cv_b)):
        g[k] = np.asarray(v, np.float32)

    wq8 = _pack_wq8(g["fuse_w"], g["fc_w"])
    wcb = _pack_wcb(g["se_w1"], g["se_w2"], g["fm_w"], g["fm_b"],
                    g["bd_w"], g["bd_b"], g["cv_w"], g["cv_b"])
    cbl = _pack_cbl(g["fuse_b"], g["fc_b"])

    x8 = np.asarray(x, np.float32).astype(F8)
    y8 = np.asarray(y, np.float32).astype(F8)
    B = x8.shape[0]

    # x: 8-row overlapping windows, stride 6, partition r*16+ic
    xpad = np.zeros((B, 16, 6 * NT + 8, W), F8)
    xpad[:, :, 1:H + 1, :] = x8
    ridx = 6 * np.arange(NT)[:, None] + np.arange(8)[None, :]
    xp = xpad[:, :, ridx, :].transpose(0, 2, 3, 1, 4).reshape(B, NT, 128, W)

    # yh: [20, 2*NT, W]: partition r4*5+c, col-block 2t+k = ypad row 6t+4k+r4
    ypad = np.zeros((B, 5, 6 * NT + 8, W), F8)
    ypad[:, :, 1:H + 1, :] = y8
    yh = ypad[:, :, ridx, :]                     # [B, 5, NT, 8, W]
    yh = yh.reshape(B, 5, NT, 2, 4, W).transpose(0, 4, 1, 2, 3, 5) \
           .reshape(B, 20, 2 * NT, W)

    # yo: [NT, 32, W]: rows i*5+c = y row 6s+i; row 30 ones; row 31 zero
    yo = np.zeros((B, NT, 32, W), F8)
    cidx = 6 * np.arange(NT)[:, None] + 1 + np.arange(6)[None, :]
    yv = ypad[:, :, cidx, :]                     # [B, 5, NT, 6, W]
    yo[:, :, 0:30, :] = yv.transpose(0, 2, 3, 1, 4).reshape(B, NT, 30, W)
    yo[:, :, 30, :] = 1.0

    in_maps = [
        {"xp": np.ascontiguousarray(xp[i]),
         "yh": np.ascontiguousarray(yh[i]),
         "yo": np.ascontiguousarray(yo[i]),
         "wq8": wq8, "wcb": wcb, "cbl": cbl}
        for i in range(B)
    ]
    return nc, in_maps


def kernel(x, y, fuse_w, fuse_b, se_w1, se_w2, bd_w, bd_b,
           fc_w, fc_b, fm_w, fm_b, cv_w, cv_b):
    global LAST_RESULT
    from concourse.bass_utils import run_bass_kernel_spmd

    nc, in_maps = prepare(x, y, fuse_w, fuse_b, se_w1, se_w2, bd_w, bd_b,
                          fc_w, fc_b, fm_w, fm_b, cv_w, cv_b)
    res = run_bass_kernel_spmd(nc, in_maps, core_ids=list(range(8)))
    LAST_RESULT = res
    outs = []
    for i in range(8):
        ot = np.asarray(res.results[i]["out"], np.float32)  # [NO, 128, W]
        full = ot.reshape(NO, 8, 16, W).transpose(2, 0, 1, 3) \
                 .reshape(16, NO * 8, W)
        outs.append(full)
    return np.stack(outs)


# revision 26
# speedup vs baseline: 1.0154x; 1.0072x over previous
"""Trainium2 Bass kernel for nn_Boundary_Enchance (dense_cnn), v3.

Pure data parallel: core i of 8 processes batch image i.  Heavy compute runs
in fp8 (e4m3) DoubleRow matmuls at 0.5 PE-cycles/row; weights are scaled x8
on the host so they stay in e4m3's normal range, and the x8 is folded into
downstream coefficients or the sigmoid scale.

Per-core pipeline:
  phase 1 (interleaved, PAIR-batched): per 8-row tile t (stride 6 rows)
    fuse_box' = relu(8*(1x1conv(y)) + 8*b) as ONE fp8 DoubleRow matmul
    (k-tiles = rows 0-3 / 4-7, K=20 each); two tiles share a 2-bank PSUM
    tile and ONE evacuation op (relu+bias+accum row sums, Act/DVE
    alternating) writes both F-halves of the [x | F] slots of the XF
    mega-tile (fp8).  One pair behind, the 3x3 conv front for strips
    (2c, 2c+1) runs as 6 fp8 DoubleRow matmuls (3 dx taps per strip;
    k-tile0 = x window, k-tile1 = F window, SAME padding via
    partial-column psum accumulation) into a 2-bank PSUM tile, evacuated
    by ONE relu+bias op into the FC mega-tile (fp8).
    Edge handling: uniform bias everywhere; tile0-row0 fixed by a memset,
    tile85's phantom bias rows are subtracted from the row-sum column.
  SE chain: row sums -> selection matmul -> gap -> MLP -> sigmoid -> se;
    4 DVE scalar_tensor_tensor ops build group-position LBM lhsT blocks.
  phase 2 (one-group lag): per 4-strip group q: four M=128 accumulating
    LBM matmuls put x8 logits in function-major 32-partition blocks of one
    PSUM bank (u | v | v' | ones); ONE sigmoid (scale=1/8) -> sg; DVE
    copies u to a twin tile at base 64 and ONE DVE min computes
    min(u, v') for all 4 strips (min(u+v,1) = v + min(u, 1-v));
    three M=128 8-row LC matmuls expand scalefactor -> 16 channels
    (bias via the sigma(160/8)=1 ones row) into 2-bank PSUM pair tiles,
    ONE evacuation per out-tile pair -> bf16 og ring; DMA out per 4 tiles.
"""

import numpy as np
import ml_dtypes

F8 = ml_dtypes.float8_e4m3
BF16 = ml_dtypes.bfloat16

H = 512
W = 512
SB = 6                      # conv strip rows
NT = (H + SB - 1) // SB     # 86 strips / fuse tiles
NPAIR = NT // 2             # 43 tile pairs
NO = 64                     # 8-row output tiles
NG = (NT + 3) // 4          # 22 tail groups (last has 2 strips)
NPIX = float(H * W)

# fp8 const block column offsets (wq8)
LFJ_C = 0          # fuse lhsT [20, 2, 128] -> 256 cols
WXD_C = 256        # conv lhsT 3 x [128, 2, 96] -> 3 * 192
WQ8_W = 256 + 3 * 192

# bf16 const block column offsets (wcb)
SEL_C = 0
W1L_C = 16
W2R_C = 32
LC_C = 160         # 3 x 128
PSB_C = LC_C + 384  # 4 x 128
LM_C = PSB_C + 512  # 4 x 128
WCB_W = LM_C + 512

_cache = {}


# ----------------------------------------------------------------------------
# host-side weight layout builders
# ----------------------------------------------------------------------------

def _fuse_lhsT(fuse_w):
    """[20, 256]: k-tile k, col r*16+oc <- 8*fuse_w[oc, c] at row r4*5+c
    (r = 4k + r4)."""
    out = np.zeros((20, 2, 128), np.float32)
    w8 = 8.0 * fuse_w[:, :, 0, 0]                # [16, 5]
    for r in range(8):
        k, r4 = divmod(r, 4)
        out[r4 * 5:r4 * 5 + 5, k, r * 16:r * 16 + 16] = w8.T
    return out.reshape(20, 256)


def _conv_lhsT(fc_w):
    """3 x [128, 192]: dx tap d: k-tile 0 = 8*fc_w[:, :16] row-Toeplitz over
    x rows, k-tile 1 = fc_w[:, 16:] over F' rows (F' = 8F)."""
    out = np.zeros((3, 128, 2, 96), np.float32)
    for dx in range(3):
        for i in range(SB):
            for ky in range(3):
                r = i + ky
                out[dx, r * 16:r * 16 + 16, 0, i * 16:i * 16 + 16] = \
                    8.0 * fc_w[:, 0:16, ky, dx].T
                out[dx, r * 16:r * 16 + 16, 1, i * 16:i * 16 + 16] = \
                    fc_w[:, 16:32, ky, dx].T
    return out.reshape(3, 128, 192)


def _lbm_static(fm_w, fm_b, bd_w, bd_b):
    """PSB_g / LM_g [4][128, 128] (bf16 inputs to the on-device stt):
    logits carry an extra x8; the sigmoid un-scales with scale=1/8.
    col layout: 8g+i = u (mask), 32+8g+i = v (boundary), 64+8g+i = v',
    96 = ones row -> 160."""
    fm_d = fm_w[1, :, 0, 0] - fm_w[0, :, 0, 0]
    bd_d = 8.0 * (bd_w[1, :, 0, 0] - bd_w[0, :, 0, 0])
    fm_bd = 8.0 * (fm_b[1] - fm_b[0])
    bd_bd = 8.0 * (bd_b[1] - bd_b[0])
    psb = np.zeros((4, 128, 128), np.float32)
    lm = np.zeros((4, 128, 128), np.float32)
    for g in range(4):
        for i in range(SB):
            for oc in range(16):
                lm[g, i * 16 + oc, 8 * g + i] = fm_d[oc]
            lm[g, 126, 8 * g + i] = fm_bd
            for c in range(5):
                psb[g, 96 + i * 5 + c, 32 + 8 * g + i] = bd_d[c]
                psb[g, 96 + i * 5 + c, 64 + 8 * g + i] = -bd_d[c]
            lm[g, 126, 32 + 8 * g + i] = bd_bd
            lm[g, 126, 64 + 8 * g + i] = -bd_bd
    lm[0, 126, 96] = 160.0
    return psb, lm


def _lc_lhsT(cv_w, cv_b):
    """3 x [128, 128]: phase p covers group rows r = 8p..8p+7; strip-in-group
    g = r//6, i = r%6: sf = v + min(u, v')."""
    w = cv_w[:, 0, 0, 0]
    b = cv_b
    out = np.zeros((3, 128, 128), np.float32)
    for p in range(3):
        for r8 in range(8):
            r = 8 * p + r8
            g, i = divmod(r, SB)
            for oc in range(16):
                m = r8 * 16 + oc
                out[p, 32 + 8 * g + i, m] = w[oc]
                out[p, 64 + 8 * g + i, m] = w[oc]
                out[p, 96, m] = b[oc]
    return out


def _sel_lhsT():
    """[128, 16]: sum valid rows 1..6 of each tile / (8 * NPIX)."""
    out = np.zeros((128, 16), np.float32)
    for r in range(1, 7):
        for fc in range(16):
            out[r * 16 + fc, fc] = 1.0 / (8.0 * NPIX)
    return out


def _w1_lhsT(se_w1):
    out = np.zeros((128, 16), np.float32)
    out[:16, :16] = se_w1.T
    return out


def _w2_lhsT(se_w2):
    """[128, 128]: se logits at out partitions 96 + i*5 + c."""
    out = np.zeros((128, 128), np.float32)
    for i in range(SB):
        for c in range(5):
            out[:16, 96 + i * 5 + c] = se_w2[c]
    return out


def _pack_wq8(fuse_w, fc_w):
    out = np.zeros((128, WQ8_W), np.float32)
    out[0:20, LFJ_C:LFJ_C + 256] = _fuse_lhsT(fuse_w)
    wxd = _conv_lhsT(fc_w)
    for d in range(3):
        out[:, WXD_C + 192 * d:WXD_C + 192 * (d + 1)] = wxd[d]
    return out.astype(F8)


def _pack_wcb(se_w1, se_w2, fm_w, fm_b, bd_w, bd_b, cv_w, cv_b):
    out = np.zeros((128, WCB_W), np.float32)
    out[:, SEL_C:SEL_C + 16] = _sel_lhsT()
    out[:, W1L_C:W1L_C + 16] = _w1_lhsT(se_w1)
    out[:, W2R_C:W2R_C + 128] = _w2_lhsT(se_w2)
    lc = _lc_lhsT(cv_w, cv_b)
    for p in range(3):
        out[:, LC_C + 128 * p:LC_C + 128 * (p + 1)] = lc[p]
    psb, lm = _lbm_static(fm_w, fm_b, bd_w, bd_b)
    for g in range(4):
        out[:, PSB_C + 128 * g:PSB_C + 128 * (g + 1)] = psb[g]
        out[:, LM_C + 128 * g:LM_C + 128 * (g + 1)] = lm[g]
    return out.astype(BF16)


def _pack_cbl(fuse_b, fc_b):
    """[128, 4] f32 bias columns: 0 = 8*fuse_b (per row-group), 1 = 8*fc_b
    (conv), 2 = tile-85 phantom row-sum correction, 3 spare."""
    out = np.zeros((128, 4), np.float32)
    for r in range(8):
        out[r * 16:r * 16 + 16, 0] = 8.0 * fuse_b
    for i in range(SB):
        out[i * 16:i * 16 + 16, 1] = 8.0 * fc_b
    relu8b = np.maximum(8.0 * fuse_b, 0.0).astype(F8).astype(np.float32)
    for r in range(3, 7):
        out[r * 16:r * 16 + 16, 2] = float(W) * relu8b
    return out


# ----------------------------------------------------------------------------
# bass graph
# ----------------------------------------------------------------------------

def _build():
    import concourse.bass as bass
    import concourse.bacc as bacc
    import concourse.tile as tile
    from concourse import mybir

    f32 = mybir.dt.float32
    bf16 = mybir.dt.bfloat16
    f8 = mybir.dt.float8e4
    AF = mybir.ActivationFunctionType
    ALU = mybir.AluOpType
    DR = mybir.MatmulPerfMode.DoubleRow

    nc = bacc.Bacc("TRN2", target_bir_lowering=False)
    xp_ext = nc.declare_dram_parameter("xp", [NT, 128, W], f8, isOutput=False)
    yh_ext = nc.declare_dram_parameter("yh", [20, 2 * NT, W], f8,
                                       isOutput=False)
    yo_ext = nc.declare_dram_parameter("yo", [NT, 32, W], f8, isOutput=False)
    wq8_ext = nc.declare_dram_parameter("wq8", [128, WQ8_W], f8,
                                        isOutput=False)
    wcb_ext = nc.declare_dram_parameter("wcb", [128, WCB_W], bf16,
                                        isOutput=False)
    cbl_ext = nc.declare_dram_parameter("cbl", [128, 4], f32, isOutput=False)
    out_ext = nc.declare_dram_parameter("out", [NO, 128, W], bf16,
                                        isOutput=True)

    NYH = 32   # yh ring slots (1024 cols each)
    NSG = 3
    NOGP = 6   # og ring pair slots (1024 cols each)

    with tile.TileContext(nc) as tc:
        with (
            tc.tile_pool(name="singles", bufs=1) as singles,
            tc.tile_pool(name="pa", bufs=2, space="PSUM") as pa,
            tc.tile_pool(name="pb", bufs=2, space="PSUM") as pb,
        ):
            wq8 = singles.tile([128, WQ8_W], f8, tag="wq8")
            nc.sync.dma_start(out=wq8[:, :], in_=wq8_ext[:, :])
            wcb = singles.tile([128, WCB_W], bf16, tag="wcb")
            nc.sync.dma_start(out=wcb[:, :], in_=wcb_ext[:, :])
            cbl = singles.tile([128, 4], f32, tag="cbl")
            nc.sync.dma_start(out=cbl[:, :], in_=cbl_ext[:, :])

            LFJ = wq8[0:20, LFJ_C:LFJ_C + 256].rearrange(
                "p (two m) -> p two m", two=2)
            WXD = [wq8[:, WXD_C + 192 * d:WXD_C + 192 * (d + 1)].rearrange(
                "p (two m) -> p two m", two=2) for d in range(3)]
            SEL = wcb[:, SEL_C:SEL_C + 16]
            W1L = wcb[:, W1L_C:W1L_C + 16]
            W2R = wcb[:, W2R_C:W2R_C + 128]
            LC = [wcb[:, LC_C + 128 * p:LC_C + 128 * (p + 1)]
                  for p in range(3)]
            PSB = [wcb[:, PSB_C + 128 * g:PSB_C + 128 * (g + 1)]
                   for g in range(4)]
            LM = [wcb[:, LM_C + 128 * g:LM_C + 128 * (g + 1)]
                  for g in range(4)]

            XF = singles.tile([128, NT * 1024 + 512], f8, tag="XF", name="XF")
            FC = singles.tile([128, NT * W], f8, tag="FC", name="FC")
            YH = singles.tile([20, NYH * 1024], f8, tag="YH", name="YH")
            OG = singles.tile([128, NOGP * 1024], bf16, tag="OG", name="OG")
            SG = [singles.tile([128, 1024], bf16, tag=f"SG{k}", name=f"SG{k}")
                  for k in range(NSG)]
            SGU = [singles.tile([128, 1024], bf16, tag=f"SGU{k}", name=f"SGU{k}")
                   for k in range(NSG)]
            Ra = singles.tile([128, NPAIR], f32, tag="Ra")
            nc.vector.memset(Ra[:, :], 0.0)
            LBMG = [singles.tile([128, 128], f8, tag=f"LBM{g}",
                                 name=f"LBM{g}") for g in range(4)]

            # ================= phase 1: fuse + conv fronts ==================
            def issue_in_dma(j):
                t0 = 8 * j
                if t0 >= NT:
                    return
                n = min(8, NT - t0)
                s0 = t0 % NYH
                nc.gpsimd.dma_start(
                    out=YH[0:20, s0 * 1024:(s0 + n) * 1024].rearrange(
                        "p (s j) -> p s j", s=2 * n),
                    in_=yh_ext[:, 2 * t0:2 * (t0 + n), :])
                nc.gpsimd.dma_start(
                    out=XF[:, t0 * 1024:(t0 + n) * 1024].rearrange(
                        "p (s j) -> p s j", s=n)[:, :, 0:W],
                    in_=xp_ext[t0:t0 + n, :, :].rearrange("s p j -> p s j"))
                nc.gpsimd.dma_start(
                    out=FC[96:128, t0 * W:(t0 + n) * W].rearrange(
                        "p (s j) -> p s j", s=n),
                    in_=yo_ext[t0:t0 + n, :, :].rearrange("s p j -> p s j"))

            def issue_fuse_pair(k):
                t0 = 2 * k
                if t0 % 8 == 0:
                    issue_in_dma(t0 // 8 + 3)
                fps = pa.tile([128, 1024], f32, tag="a")
                for h in range(2):
                    t = t0 + h
                    s = t % NYH
                    rhs = YH[0:20, s * 1024:(s + 1) * 1024].rearrange(
                        "p (two j) -> p two j", two=2)
                    nc.tensor.matmul(fps[:, h * W:(h + 1) * W], lhsT=LFJ,
                                     rhs=rhs, start=True, stop=True,
                                     perf_mode=DR)
                # one evac for both halves -> F-halves of XF slots t0, t0+1
                dst = XF[:, t0 * 1024 + W:t0 * 1024 + W + 2048].rearrange(
                    "p (s j) -> p s j", s=2)[:, :, 0:W]
                nc.vector.tensor_scalar(out=dst, in0=fps[:, :],
                                        scalar1=cbl[:, 0:1],
                                        scalar2=0.0,
                                        op0=ALU.add, op1=ALU.max,
                                        accum_out=Ra[:, k:k + 1])
                if k == 0:
                    # tile0 row0 is image row -1: kill its bias-only relu
                    nc.vector.memset(XF[0:16, W:2 * W], 0.0)

            def issue_front_pair(c):
                cps = pb.tile([96, 1024], f32, tag="b")
                for h in range(2):
                    s = 2 * c + h
                    v = XF[:, s * 1024:(s + 1) * 1024].rearrange(
                        "p (two j) -> p two j", two=2)
                    o = h * W
                    nc.tensor.matmul(cps[:, o:o + W], lhsT=WXD[1],
                                     rhs=v[:, :, 0:W],
                                     start=True, stop=False, perf_mode=DR)
                    nc.tensor.matmul(cps[:, o + 1:o + W], lhsT=WXD[0],
                                     rhs=v[:, :, 0:W - 1],
                                     start=False, stop=False, perf_mode=DR)
                    nc.tensor.matmul(cps[:, o:o + W - 1], lhsT=WXD[2],
                                     rhs=v[:, :, 1:W],
                                     start=False, stop=True, perf_mode=DR)
                dst = FC[0:96, 2 * c * W:(2 * c + 2) * W]
                nc.scalar.activation(out=dst, in_=cps[:, :], func=AF.Relu,
                                     bias=cbl[0:96, 1:2])

            # ================= SE chain =====================================
            def issue_se():
                # tile-85 phantom bias rows leaked into Ra col 42: subtract
                nc.vector.tensor_scalar(out=Ra[:, NPAIR - 1:NPAIR],
                                        in0=Ra[:, NPAIR - 1:NPAIR],
                                        scalar1=cbl[:, 2:3], scalar2=0.0,
                                        op0=ALU.subtract, op1=ALU.add)
                Rbf = singles.tile([128, NPAIR], bf16, tag="Rbf")
                nc.vector.tensor_copy(out=Rbf[:, :], in_=Ra[:, :])
                gps = pb.tile([16, NPAIR], f32, tag="b")
                nc.tensor.matmul(gps[:, :], lhsT=SEL, rhs=Rbf[:, :],
                                 start=True, stop=True)
                gap_f = singles.tile([16, 1], f32, tag="gapf")
                nc.vector.reduce_sum(out=gap_f[:, :], in_=gps[:, :],
                                     axis=mybir.AxisListType.X)
                gap_bf = singles.tile([128, 1], bf16, tag="gap")
                nc.vector.memset(gap_bf[:, :], 0.0)
                nc.vector.tensor_copy(out=gap_bf[0:16, :], in_=gap_f[:, :])
                hps = pb.tile([16, 1], f32, tag="b")
                nc.tensor.matmul(hps[:, :], lhsT=W1L, rhs=gap_bf[:, :],
                                 start=True, stop=True)
                h_bf = singles.tile([128, 1], bf16, tag="hbf")
                nc.vector.memset(h_bf[:, :], 0.0)
                nc.scalar.activation(out=h_bf[0:16, :], in_=hps[:, :],
                                     func=AF.Relu)
                sps = pb.tile([128, 1], f32, tag="b")
                nc.tensor.matmul(sps[:, :], lhsT=W2R, rhs=h_bf[:, :],
                                 start=True, stop=True)
                se_bc = singles.tile([128, 1], f32, tag="sebc")
                nc.scalar.activation(out=se_bc[:, :], in_=sps[:, :],
                                     func=AF.Sigmoid)
                for g in range(4):
                    nc.vector.scalar_tensor_tensor(
                        out=LBMG[g][:, :], in0=PSB[g], scalar=se_bc[:, :],
                        in1=LM[g], op0=ALU.mult, op1=ALU.add)

            # ================= phase 2: tails ===============================
            cvt = {}
            p2ctr = [0]

            def p2tile(name):
                i = p2ctr[0]
                p2ctr[0] += 1
                pool = pa if i % 2 == 0 else pb
                return pool.tile([128, 1024], f32,
                                 tag=("a" if i % 2 == 0 else "b"), name=name)

            def issue_head_pair(j):
                mb = p2tile(f"mb{j}")
                for h in range(2):
                    q = 2 * j + h
                    ns = min(4, NT - 4 * q)
                    for g in range(ns):
                        u = 4 * q + g
                        nc.tensor.matmul(mb[:, h * W:(h + 1) * W],
                                         lhsT=LBMG[g][:, :],
                                         rhs=FC[:, u * W:(u + 1) * W],
                                         start=(g == 0), stop=(g == ns - 1))
                sg = SG[j % NSG]
                sgu = SGU[j % NSG]
                nc.scalar.activation(out=sg[:, :], in_=mb[:, :],
                                     func=AF.Sigmoid, scale=0.125)
                nc.vector.tensor_copy(out=sgu[64:96, :], in_=sg[0:32, :])
                nc.vector.tensor_tensor(out=sg[64:96, :], in0=sgu[64:96, :],
                                        in1=sg[64:96, :], op=ALU.min)

            def issue_head_pair_split(j):
                mb = pb.tile([128, 1024], f32, tag="b", name=f"mb{j}")
                sg = SG[j % NSG]
                sgu = SGU[j % NSG]
                for h in range(2):
                    q = 2 * j + h
                    ns = min(4, NT - 4 * q)
                    for g in range(ns):
                        u = 4 * q + g
                        nc.tensor.matmul(mb[:, h * W:(h + 1) * W],
                                         lhsT=LBMG[g][:, :],
                                         rhs=FC[:, u * W:(u + 1) * W],
                                         start=(g == 0), stop=(g == ns - 1))
                    hv = slice(h * W, (h + 1) * W)
                    nc.scalar.activation(out=sg[:, hv], in_=mb[:, hv],
                                         func=AF.Sigmoid, scale=0.125)
                nc.vector.tensor_copy(out=sgu[64:96, :], in_=sg[0:32, :])
                nc.vector.tensor_tensor(out=sg[64:96, :], in0=sgu[64:96, :],
                                        in1=sg[64:96, :], op=ALU.min)

            def issue_lc(q):
                sg = SG[(q // 2) % NSG][:, (q % 2) * W:(q % 2 + 1) * W]
                np_ = 3 if q < NG - 1 else 1
                for p in range(np_):
                    tau = 3 * q + p
                    pi, h = divmod(tau, 2)
                    if h == 0:
                        cvt[pi] = p2tile(f"cv{pi}")
                    ops = cvt[pi]
                    nc.tensor.matmul(ops[:, h * W:(h + 1) * W], lhsT=LC[p],
                                     rhs=sg, start=True, stop=True)
                    if h == 1:
                        dst = OG[:, (pi % NOGP) * 1024:
                                 (pi % NOGP + 1) * 1024]
                        if pi % 2 == 0:
                            nc.scalar.activation(out=dst, in_=ops[:, :],
                                                 func=AF.Copy)
                        else:
                            nc.vector.tensor_copy(out=dst, in_=ops[:, :])
                        del cvt[pi]
                        if pi % 3 == 2 or pi == 31:
                            p0 = pi - (pi % 3)
                            t0 = 2 * p0
                            n = 2 * (pi - p0 + 1)
                            c0 = (p0 % NOGP) * 1024
                            nc.sync.dma_start(
                                out=out_ext[t0:t0 + n, :, :].rearrange(
                                    "s p j -> p s j"),
                                in_=OG[:, c0:c0 + n * W].rearrange(
                                    "p (s j) -> p s j", s=n))

            for j in range(3):
                issue_in_dma(j)
            for k in range(NPAIR + 2):
                if k < NPAIR:
                    issue_fuse_pair(k)
                if k >= 2:
                    issue_front_pair(k - 2)
            issue_se()
            NJ = (NG + 1) // 2
            for j in range(NJ + 1):
                if j < NJ:
                    issue_head_pair(j)
                if j >= 1:
                    issue_lc(2 * (j - 1))
                    issue_lc(2 * (j - 1) + 1)
    nc.compile()
    return nc


# ----------------------------------------------------------------------------
# entry point
# ----------------------------------------------------------------------------

LAST_RESULT = None


def prepare(x, y, fuse_w, fuse_b, se_w1, se_w2, bd_w, bd_b,
            fc_w, fc_b, fm_w, fm_b, cv_w, cv_b):
    if "nc" not in _cache:
        _cache["nc"] = _build()
    nc = _cache["nc"]

    g = {}
    for k, v in (("fuse_w", fuse_w), ("fuse_b", fuse_b), ("se_w1", se_w1),
                 ("se_w2", se_w2), ("bd_w", bd_w), ("bd_b", bd_b),
                 ("fc_w", fc_w), ("fc_b", fc_b), ("fm_w", fm_w),
                 ("fm_b", fm_b), ("cv_w", cv_w), ("cv_b", cv_b)):
        g[k] = np.asarray(v, np.float32)

    wq8 = _pack_wq8(g["fuse_w"], g["fc_w"])
    wcb = _pack_wcb(g["se_w1"], g["se_w2"], g["fm_w"], g["fm_b"],
                    g["bd_w"], g["bd_b"], g["cv_w"], g["cv_b"])
    cbl = _pack_cbl(g["fuse_b"], g["fc_b"])

    x8 = np.asarray(x, np.float32).astype(F8)
    y8 = np.asarray(y, np.float32).astype(F8)
    B = x8.shape[0]

    # x: 8-row overlapping windows, stride 6, partition r*16+ic
    xpad = np.zeros((B, 16, 6 * NT + 8, W), F8)
    xpad[:, :, 1:H + 1, :] = x8
    ridx = 6 * np.arange(NT)[:, None] + np.arange(8)[None, :]
    xp = xpad[:, :, ridx, :].transpose(0, 2, 3, 1, 4).reshape(B, NT, 128, W)

    # yh: [20, 2*NT, W]: partition r4*5+c, col-block 2t+k = ypad row 6t+4k+r4
    ypad = np.zeros((B, 5, 6 * NT + 8, W), F8)
    ypad[:, :, 1:H + 1, :] = y8
    yh = ypad[:, :, ridx, :]                     # [B, 5, NT, 8, W]
    yh = yh.reshape(B, 5, NT, 2, 4, W).transpose(0, 4, 1, 2, 3, 5) \
           .reshape(B, 20, 2 * NT, W)

    # yo: [NT, 32, W]: rows i*5+c = y row 6s+i; row 30 ones; row 31 zero
    yo = np.zeros((B, NT, 32, W), F8)
    cidx = 6 * np.arange(NT)[:, None] + 1 + np.arange(6)[None, :]
    yv = ypad[:, :, cidx, :]                     # [B, 5, NT, 6, W]
    yo[:, :, 0:30, :] = yv.transpose(0, 2, 3, 1, 4).reshape(B, NT, 30, W)
    yo[:, :, 30, :] = 1.0

    in_maps = [
        {"xp": np.ascontiguousarray(xp[i]),
         "yh": np.ascontiguousarray(yh[i]),
         "yo": np.ascontiguousarray(yo[i]),
         "wq8": wq8, "wcb": wcb, "cbl": cbl}
        for i in range(B)
    ]
    return nc, in_maps


def kernel(x, y, fuse_w, fuse_b, se_w1, se_w2, bd_w, bd_b,
           fc_w, fc_b, fm_w, fm_b, cv_w, cv_b):
    global LAST_RESULT
    from concourse.bass_utils import run_bass_kernel_spmd

    nc, in_maps = prepare(x, y, fuse_w, fuse_b, se_w1, se_w2, bd_w, bd_b,
                          fc_w, fc_b, fm_w, fm_b, cv_w, cv_b)
    res = run_bass_kernel_spmd(nc, in_maps, core_ids=list(range(8)))
    LAST_RESULT = res
    outs = []
    for i in range(8):
        ot = np.asarray(res.results[i]["out"], np.float32)  # [NO, 128, W]
        full = ot.reshape(NO, 8, 16, W).transpose(2, 0, 1, 3) \
                 .reshape(16, NO * 8, W)
        outs.append(full)
    return np.stack(outs)


# revision 31
# speedup vs baseline: 1.0350x; 1.0193x over previous
"""Trainium2 Bass kernel for nn_Boundary_Enchance (dense_cnn), v3.

Pure data parallel: core i of 8 processes batch image i.  Heavy compute runs
in fp8 (e4m3) DoubleRow matmuls at 0.5 PE-cycles/row; weights are scaled x8
on the host so they stay in e4m3's normal range, and the x8 is folded into
downstream coefficients or the sigmoid scale.

Per-core pipeline:
  phase 1 (interleaved, PAIR-batched): per 8-row tile t (stride 6 rows)
    fuse_box' = relu(8*(1x1conv(y)) + 8*b) as ONE fp8 DoubleRow matmul
    (k-tiles = rows 0-3 / 4-7, K=20 each); two tiles share a 2-bank PSUM
    tile and ONE evacuation op (relu+bias+accum row sums, Act/DVE
    alternating) writes both F-halves of the [x | F] slots of the XF
    mega-tile (fp8).  One pair behind, the 3x3 conv front for strips
    (2c, 2c+1) runs as 6 fp8 DoubleRow matmuls (3 dx taps per strip;
    k-tile0 = x window, k-tile1 = F window, SAME padding via
    partial-column psum accumulation) into a 2-bank PSUM tile, evacuated
    by ONE relu+bias op into the FC mega-tile (fp8).
    Edge handling: uniform bias everywhere; tile0-row0 fixed by a memset,
    tile85's phantom bias rows are subtracted from the row-sum column.
  SE chain: row sums -> selection matmul -> gap -> MLP -> sigmoid -> se;
    4 DVE scalar_tensor_tensor ops build group-position LBM lhsT blocks.
  phase 2 (one-group lag): per 4-strip group q: four M=128 accumulating
    LBM matmuls put x8 logits in function-major 32-partition blocks of one
    PSUM bank (u | v | v' | ones); ONE sigmoid (scale=1/8) -> sg; DVE
    copies u to a twin tile at base 64 and ONE DVE min computes
    min(u, v') for all 4 strips (min(u+v,1) = v + min(u, 1-v));
    three M=128 8-row LC matmuls expand scalefactor -> 16 channels
    (bias via the sigma(160/8)=1 ones row) into 2-bank PSUM pair tiles,
    ONE evacuation per out-tile pair -> bf16 og ring; DMA out per 4 tiles.
"""

import numpy as np
import ml_dtypes

F8 = ml_dtypes.float8_e4m3
BF16 = ml_dtypes.bfloat16

H = 512
W = 512
SB = 6                      # conv strip rows
NT = (H + SB - 1) // SB     # 86 strips / fuse tiles
NPAIR = NT // 2             # 43 tile pairs
NO = 64                     # 8-row output tiles
NG = (NT + 3) // 4          # 22 tail groups (last has 2 strips)
NPIX = float(H * W)

# fp8 const block column offsets (wq8)
LFJ_C = 0          # fuse lhsT [20, 2, 128] -> 256 cols
WXD_C = 256        # conv lhsT 3 x [128, 2, 96] -> 3 * 192
WQ8_W = 256 + 3 * 192

# bf16 const block column offsets (wcb)
SEL_C = 0
W1L_C = 16
W2R_C = 32
LC_C = 160         # 3 x 128
PSB_C = LC_C + 384  # 4 x 128
LM_C = PSB_C + 512  # 4 x 128
WCB_W = LM_C + 512

_cache = {}


# ----------------------------------------------------------------------------
# host-side weight layout builders
# ----------------------------------------------------------------------------

def _fuse_lhsT(fuse_w):
    """[20, 256]: k-tile k, col r*16+oc <- 8*fuse_w[oc, c] at row r4*5+c
    (r = 4k + r4)."""
    out = np.zeros((20, 2, 128), np.float32)
    w8 = 8.0 * fuse_w[:, :, 0, 0]                # [16, 5]
    for r in range(8):
        k, r4 = divmod(r, 4)
        out[r4 * 5:r4 * 5 + 5, k, r * 16:r * 16 + 16] = w8.T
    return out.reshape(20, 256)


def _conv_lhsT(fc_w):
    """3 x [128, 192]: dx tap d: k-tile 0 = 8*fc_w[:, :16] row-Toeplitz over
    x rows, k-tile 1 = fc_w[:, 16:] over F' rows (F' = 8F)."""
    out = np.zeros((3, 128, 2, 96), np.float32)
    for dx in range(3):
        for i in range(SB):
            for ky in range(3):
                r = i + ky
                out[dx, r * 16:r * 16 + 16, 0, i * 16:i * 16 + 16] = \
                    8.0 * fc_w[:, 0:16, ky, dx].T
                out[dx, r * 16:r * 16 + 16, 1, i * 16:i * 16 + 16] = \
                    fc_w[:, 16:32, ky, dx].T
    return out.reshape(3, 128, 192)


def _lbm_static(fm_w, fm_b, bd_w, bd_b):
    """PSB_g / LM_g [4][128, 128] (bf16 inputs to the on-device stt):
    logits carry an extra x8; the sigmoid un-scales with scale=1/8.
    col layout: 8g+i = u (mask), 32+8g+i = v (boundary), 64+8g+i = v',
    96 = ones row -> 160."""
    fm_d = fm_w[1, :, 0, 0] - fm_w[0, :, 0, 0]
    bd_d = 8.0 * (bd_w[1, :, 0, 0] - bd_w[0, :, 0, 0])
    fm_bd = 8.0 * (fm_b[1] - fm_b[0])
    bd_bd = 8.0 * (bd_b[1] - bd_b[0])
    psb = np.zeros((4, 128, 128), np.float32)
    lm = np.zeros((4, 128, 128), np.float32)
    for g in range(4):
        for i in range(SB):
            for oc in range(16):
                lm[g, i * 16 + oc, 8 * g + i] = fm_d[oc]
            lm[g, 126, 8 * g + i] = fm_bd
            for c in range(5):
                psb[g, 96 + i * 5 + c, 32 + 8 * g + i] = bd_d[c]
                psb[g, 96 + i * 5 + c, 64 + 8 * g + i] = -bd_d[c]
            lm[g, 126, 32 + 8 * g + i] = bd_bd
            lm[g, 126, 64 + 8 * g + i] = -bd_bd
    lm[0, 126, 96] = 160.0
    return psb, lm


def _lc_lhsT(cv_w, cv_b):
    """3 x [128, 128]: phase p covers group rows r = 8p..8p+7; strip-in-group
    g = r//6, i = r%6: sf = v + min(u, v')."""
    w = cv_w[:, 0, 0, 0]
    b = cv_b
    out = np.zeros((3, 128, 128), np.float32)
    for p in range(3):
        for r8 in range(8):
            r = 8 * p + r8
            g, i = divmod(r, SB)
            for oc in range(16):
                m = r8 * 16 + oc
                out[p, 32 + 8 * g + i, m] = w[oc]
                out[p, 64 + 8 * g + i, m] = w[oc]
                out[p, 96, m] = b[oc]
    return out


def _sel_lhsT():
    """[128, 16]: sum valid rows 1..6 of each tile / (8 * NPIX)."""
    out = np.zeros((128, 16), np.float32)
    for r in range(1, 7):
        for fc in range(16):
            out[r * 16 + fc, fc] = 1.0 / (8.0 * NPIX)
    return out


def _w1_lhsT(se_w1):
    out = np.zeros((128, 16), np.float32)
    out[:16, :16] = se_w1.T
    return out


def _w2_lhsT(se_w2):
    """[128, 128]: se logits at out partitions 96 + i*5 + c."""
    out = np.zeros((128, 128), np.float32)
    for i in range(SB):
        for c in range(5):
            out[:16, 96 + i * 5 + c] = se_w2[c]
    return out


def _pack_wq8(fuse_w, fc_w):
    out = np.zeros((128, WQ8_W), np.float32)
    out[0:20, LFJ_C:LFJ_C + 256] = _fuse_lhsT(fuse_w)
    wxd = _conv_lhsT(fc_w)
    for d in range(3):
        out[:, WXD_C + 192 * d:WXD_C + 192 * (d + 1)] = wxd[d]
    return out.astype(F8)


def _pack_wcb(se_w1, se_w2, fm_w, fm_b, bd_w, bd_b, cv_w, cv_b):
    out = np.zeros((128, WCB_W), np.float32)
    out[:, SEL_C:SEL_C + 16] = _sel_lhsT()
    out[:, W1L_C:W1L_C + 16] = _w1_lhsT(se_w1)
    out[:, W2R_C:W2R_C + 128] = _w2_lhsT(se_w2)
    lc = _lc_lhsT(cv_w, cv_b)
    for p in range(3):
        out[:, LC_C + 128 * p:LC_C + 128 * (p + 1)] = lc[p]
    psb, lm = _lbm_static(fm_w, fm_b, bd_w, bd_b)
    for g in range(4):
        out[:, PSB_C + 128 * g:PSB_C + 128 * (g + 1)] = psb[g]
        out[:, LM_C + 128 * g:LM_C + 128 * (g + 1)] = lm[g]
    return out.astype(BF16)


def _pack_cbl(fuse_b, fc_b):
    """[128, 4] f32 bias columns: 0 = 8*fuse_b (per row-group), 1 = 8*fc_b
    (conv), 2 = tile-85 phantom row-sum correction, 3 spare."""
    out = np.zeros((128, 4), np.float32)
    for r in range(8):
        out[r * 16:r * 16 + 16, 0] = 8.0 * fuse_b
    for i in range(SB):
        out[i * 16:i * 16 + 16, 1] = 8.0 * fc_b
    relu8b = np.maximum(8.0 * fuse_b, 0.0).astype(F8).astype(np.float32)
    for r in range(3, 7):
        out[r * 16:r * 16 + 16, 2] = float(W) * relu8b
    return out


# ----------------------------------------------------------------------------
# bass graph
# ----------------------------------------------------------------------------

def _build():
    import concourse.bass as bass
    import concourse.bacc as bacc
    import concourse.tile as tile
    from concourse import mybir

    f32 = mybir.dt.float32
    bf16 = mybir.dt.bfloat16
    f8 = mybir.dt.float8e4
    AF = mybir.ActivationFunctionType
    ALU = mybir.AluOpType
    DR = mybir.MatmulPerfMode.DoubleRow

    nc = bacc.Bacc("TRN2", target_bir_lowering=False)
    xp_ext = nc.declare_dram_parameter("xp", [NT, 128, W], f8, isOutput=False)
    yh_ext = nc.declare_dram_parameter("yh", [20, 2 * NT, W], f8,
                                       isOutput=False)
    yo_ext = nc.declare_dram_parameter("yo", [NT, 32, W], f8, isOutput=False)
    wq8_ext = nc.declare_dram_parameter("wq8", [128, WQ8_W], f8,
                                        isOutput=False)
    wcb_ext = nc.declare_dram_parameter("wcb", [128, WCB_W], bf16,
                                        isOutput=False)
    cbl_ext = nc.declare_dram_parameter("cbl", [128, 4], f32, isOutput=False)
    out_ext = nc.declare_dram_parameter("out", [NO, 128, W], bf16,
                                        isOutput=True)

    NYH = 32   # yh ring slots (1024 cols each)
    NSG = 3
    NOGP = 6   # og ring pair slots (1024 cols each)

    with tile.TileContext(nc) as tc:
        with (
            tc.tile_pool(name="singles", bufs=1) as singles,
            tc.tile_pool(name="pa", bufs=2, space="PSUM") as pa,
            tc.tile_pool(name="pb", bufs=2, space="PSUM") as pb,
        ):
            wq8 = singles.tile([128, WQ8_W], f8, tag="wq8")
            nc.sync.dma_start(out=wq8[:, :], in_=wq8_ext[:, :])
            wcb = singles.tile([128, WCB_W], bf16, tag="wcb")
            nc.sync.dma_start(out=wcb[:, :], in_=wcb_ext[:, :])
            cbl = singles.tile([128, 4], f32, tag="cbl")
            nc.sync.dma_start(out=cbl[:, :], in_=cbl_ext[:, :])

            LFJ = wq8[0:20, LFJ_C:LFJ_C + 256].rearrange(
                "p (two m) -> p two m", two=2)
            WXD = [wq8[:, WXD_C + 192 * d:WXD_C + 192 * (d + 1)].rearrange(
                "p (two m) -> p two m", two=2) for d in range(3)]
            SEL = wcb[:, SEL_C:SEL_C + 16]
            W1L = wcb[:, W1L_C:W1L_C + 16]
            W2R = wcb[:, W2R_C:W2R_C + 128]
            LC = [wcb[:, LC_C + 128 * p:LC_C + 128 * (p + 1)]
                  for p in range(3)]
            PSB = [wcb[:, PSB_C + 128 * g:PSB_C + 128 * (g + 1)]
                   for g in range(4)]
            LM = [wcb[:, LM_C + 128 * g:LM_C + 128 * (g + 1)]
                  for g in range(4)]

            XF = singles.tile([128, NT * 1024 + 512], f8, tag="XF", name="XF")
            FC = singles.tile([128, NT * W], f8, tag="FC", name="FC")
            YH = singles.tile([20, NYH * 1024], f8, tag="YH", name="YH")
            OG = singles.tile([128, NOGP * 1024], bf16, tag="OG", name="OG")
            SG = [singles.tile([128, 1024], bf16, tag=f"SG{k}", name=f"SG{k}")
                  for k in range(NSG)]
            SGU = [singles.tile([128, 1024], bf16, tag=f"SGU{k}", name=f"SGU{k}")
                   for k in range(NSG)]
            Ra = singles.tile([128, NPAIR], f32, tag="Ra")
            nc.vector.memset(Ra[:, :], 0.0)
            LBMG = [singles.tile([128, 256], f8, tag=f"LBM{g}",
                                 name=f"LBM{g}") for g in range(2)]

            # ================= phase 1: fuse + conv fronts ==================
            def issue_in_dma(j):
                t0 = 8 * j
                if t0 >= NT:
                    return
                n = min(8, NT - t0)
                s0 = t0 % NYH
                nc.gpsimd.dma_start(
                    out=YH[0:20, s0 * 1024:(s0 + n) * 1024].rearrange(
                        "p (s j) -> p s j", s=2 * n),
                    in_=yh_ext[:, 2 * t0:2 * (t0 + n), :])
                nc.gpsimd.dma_start(
                    out=XF[:, t0 * 1024:(t0 + n) * 1024].rearrange(
                        "p (s j) -> p s j", s=n)[:, :, 0:W],
                    in_=xp_ext[t0:t0 + n, :, :].rearrange("s p j -> p s j"))
                nc.gpsimd.dma_start(
                    out=FC[96:128, t0 * W:(t0 + n) * W].rearrange(
                        "p (s j) -> p s j", s=n),
                    in_=yo_ext[t0:t0 + n, :, :].rearrange("s p j -> p s j"))

            def issue_fuse_pair(k):
                t0 = 2 * k
                if t0 % 8 == 0:
                    issue_in_dma(t0 // 8 + 3)
                fps = pa.tile([128, 1024], f32, tag="a")
                for h in range(2):
                    t = t0 + h
                    s = t % NYH
                    rhs = YH[0:20, s * 1024:(s + 1) * 1024].rearrange(
                        "p (two j) -> p two j", two=2)
                    nc.tensor.matmul(fps[:, h * W:(h + 1) * W], lhsT=LFJ,
                                     rhs=rhs, start=True, stop=True,
                                     perf_mode=DR)
                # one evac for both halves -> F-halves of XF slots t0, t0+1
                dst = XF[:, t0 * 1024 + W:t0 * 1024 + W + 2048].rearrange(
                    "p (s j) -> p s j", s=2)[:, :, 0:W]
                nc.vector.tensor_scalar(out=dst, in0=fps[:, :],
                                        scalar1=cbl[:, 0:1],
                                        scalar2=0.0,
                                        op0=ALU.add, op1=ALU.max,
                                        accum_out=Ra[:, k:k + 1])
                if k == 0:
                    # tile0 row0 is image row -1: kill its bias-only relu
                    nc.vector.memset(XF[0:16, W:2 * W], 0.0)

            def issue_front_pair(c):
                cps = pb.tile([96, 1024], f32, tag="b")
                for h in range(2):
                    s = 2 * c + h
                    v = XF[:, s * 1024:(s + 1) * 1024].rearrange(
                        "p (two j) -> p two j", two=2)
                    o = h * W
                    nc.tensor.matmul(cps[:, o:o + W], lhsT=WXD[1],
                                     rhs=v[:, :, 0:W],
                                     start=True, stop=False, perf_mode=DR)
                    nc.tensor.matmul(cps[:, o + 1:o + W], lhsT=WXD[0],
                                     rhs=v[:, :, 0:W - 1],
                                     start=False, stop=False, perf_mode=DR)
                    nc.tensor.matmul(cps[:, o:o + W - 1], lhsT=WXD[2],
                                     rhs=v[:, :, 1:W],
                                     start=False, stop=True, perf_mode=DR)
                dst = FC[0:96, 2 * c * W:(2 * c + 2) * W]
                nc.scalar.activation(out=dst, in_=cps[:, :], func=AF.Relu,
                                     bias=cbl[0:96, 1:2])

            # ================= SE chain =====================================
            def issue_se():
                # tile-85 phantom bias rows leaked into Ra col 42: subtract
                nc.vector.tensor_scalar(out=Ra[:, NPAIR - 1:NPAIR],
                                        in0=Ra[:, NPAIR - 1:NPAIR],
                                        scalar1=cbl[:, 2:3], scalar2=0.0,
                                        op0=ALU.subtract, op1=ALU.add)
                Rbf = singles.tile([128, NPAIR], bf16, tag="Rbf")
                nc.vector.tensor_copy(out=Rbf[:, :], in_=Ra[:, :])
                gps = pb.tile([16, NPAIR], f32, tag="b")
                nc.tensor.matmul(gps[:, :], lhsT=SEL, rhs=Rbf[:, :],
                                 start=True, stop=True)
                gap_f = singles.tile([16, 1], f32, tag="gapf")
                nc.vector.reduce_sum(out=gap_f[:, :], in_=gps[:, :],
                                     axis=mybir.AxisListType.X)
                gap_bf = singles.tile([128, 1], bf16, tag="gap")
                nc.vector.memset(gap_bf[:, :], 0.0)
                nc.vector.tensor_copy(out=gap_bf[0:16, :], in_=gap_f[:, :])
                hps = pb.tile([16, 1], f32, tag="b")
                nc.tensor.matmul(hps[:, :], lhsT=W1L, rhs=gap_bf[:, :],
                                 start=True, stop=True)
                h_bf = singles.tile([128, 1], bf16, tag="hbf")
                nc.vector.memset(h_bf[:, :], 0.0)
                nc.scalar.activation(out=h_bf[0:16, :], in_=hps[:, :],
                                     func=AF.Relu)
                sps = pb.tile([128, 1], f32, tag="b")
                nc.tensor.matmul(sps[:, :], lhsT=W2R, rhs=h_bf[:, :],
                                 start=True, stop=True)
                se_bc = singles.tile([128, 1], f32, tag="sebc")
                nc.scalar.activation(out=se_bc[:, :], in_=sps[:, :],
                                     func=AF.Sigmoid)
                for g in range(2):
                    nc.vector.scalar_tensor_tensor(
                        out=LBMG[g][:, :],
                        in0=wcb[:, PSB_C + 256 * g:PSB_C + 256 * (g + 1)],
                        scalar=se_bc[:, :],
                        in1=wcb[:, LM_C + 256 * g:LM_C + 256 * (g + 1)],
                        op0=ALU.mult, op1=ALU.add)

            # ================= phase 2: tails ===============================
            cvt = {}
            p2ctr = [0]

            def p2tile(name):
                i = p2ctr[0]
                p2ctr[0] += 1
                pool = pa if i % 2 == 0 else pb
                return pool.tile([128, 1024], f32,
                                 tag=("a" if i % 2 == 0 else "b"), name=name)

            def issue_head_pair(j):
                mb = p2tile(f"mb{j}")
                for h in range(2):
                    q = 2 * j + h
                    ns = min(4, NT - 4 * q)
                    nm = ns // 2
                    for g2 in range(nm):
                        u0 = 4 * q + 2 * g2
                        nc.tensor.matmul(
                            mb[:, h * W:(h + 1) * W],
                            lhsT=LBMG[g2][:, :].rearrange(
                                "p (two m) -> p two m", two=2),
                            rhs=FC[:, u0 * W:(u0 + 2) * W].rearrange(
                                "p (two j) -> p two j", two=2),
                            start=(g2 == 0), stop=(g2 == nm - 1),
                            perf_mode=DR)
                sg = SG[j % NSG]
                sgu = SGU[j % NSG]
                nc.scalar.activation(out=sg[:, :], in_=mb[:, :],
                                     func=AF.Sigmoid, scale=0.125)
                nc.vector.tensor_copy(out=sgu[64:96, :], in_=sg[0:32, :])
                nc.vector.tensor_tensor(out=sg[64:96, :], in0=sgu[64:96, :],
                                        in1=sg[64:96, :], op=ALU.min)

            def issue_head_pair_split(j):
                mb = pb.tile([128, 1024], f32, tag="b", name=f"mb{j}")
                sg = SG[j % NSG]
                sgu = SGU[j % NSG]
                for h in range(2):
                    q = 2 * j + h
                    ns = min(4, NT - 4 * q)
                    for g in range(ns):
                        u = 4 * q + g
                        nc.tensor.matmul(mb[:, h * W:(h + 1) * W],
                                         lhsT=LBMG[g][:, :],
                                         rhs=FC[:, u * W:(u + 1) * W],
                                         start=(g == 0), stop=(g == ns - 1))
                    hv = slice(h * W, (h + 1) * W)
                    nc.scalar.activation(out=sg[:, hv], in_=mb[:, hv],
                                         func=AF.Sigmoid, scale=0.125)
                nc.vector.tensor_copy(out=sgu[64:96, :], in_=sg[0:32, :])
                nc.vector.tensor_tensor(out=sg[64:96, :], in0=sgu[64:96, :],
                                        in1=sg[64:96, :], op=ALU.min)

            def issue_lc(q):
                sg = SG[(q // 2) % NSG][:, (q % 2) * W:(q % 2 + 1) * W]
                np_ = 3 if q < NG - 1 else 1
                for p in range(np_):
                    tau = 3 * q + p
                    pi, h = divmod(tau, 2)
                    if h == 0:
                        cvt[pi] = p2tile(f"cv{pi}")
                    ops = cvt[pi]
                    nc.tensor.matmul(ops[:, h * W:(h + 1) * W], lhsT=LC[p],
                                     rhs=sg, start=True, stop=True)
                    if h == 1:
                        dst = OG[:, (pi % NOGP) * 1024:
                                 (pi % NOGP + 1) * 1024]
                        if pi % 2 == 0:
                            nc.scalar.activation(out=dst, in_=ops[:, :],
                                                 func=AF.Copy)
                        else:
                            nc.vector.tensor_copy(out=dst, in_=ops[:, :])
                        del cvt[pi]
                        if pi % 3 == 2 or pi == 31:
                            p0 = pi - (pi % 3)
                            t0 = 2 * p0
                            n = 2 * (pi - p0 + 1)
                            c0 = (p0 % NOGP) * 1024
                            nc.sync.dma_start(
                                out=out_ext[t0:t0 + n, :, :].rearrange(
                                    "s p j -> p s j"),
                                in_=OG[:, c0:c0 + n * W].rearrange(
                                    "p (s j) -> p s j", s=n))

            for j in range(3):
                issue_in_dma(j)
            for k in range(NPAIR + 2):
                if k < NPAIR:
                    issue_fuse_pair(k)
                if k >= 2:
                    issue_front_pair(k - 2)
            issue_se()
            NJ = (NG + 1) // 2
            for j in range(NJ + 1):
                if j < NJ:
                    issue_head_pair(j)
                if j >= 1:
                    issue_lc(2 * (j - 1))
                    issue_lc(2 * (j - 1) + 1)
    nc.compile()
    return nc


# ----------------------------------------------------------------------------
# entry point
# ----------------------------------------------------------------------------

LAST_RESULT = None


def prepare(x, y, fuse_w, fuse_b, se_w1, se_w2, bd_w, bd_b,
            fc_w, fc_b, fm_w, fm_b, cv_w, cv_b):
    if "nc" not in _cache:
        _cache["nc"] = _build()
    nc = _cache["nc"]

    g = {}
    for k, v in (("fuse_w", fuse_w), ("fuse_b", fuse_b), ("se_w1", se_w1),
                 ("se_w2", se_w2), ("bd_w", bd_w), ("bd_b", bd_b),
                 ("fc_w", fc_w), ("fc_b", fc_b), ("fm_w", fm_w),
                 ("fm_b", fm_b), ("cv_w", cv_w), ("cv_b", cv_b)):
        g[k] = np.asarray(v, np.float32)

    wq8 = _pack_wq8(g["fuse_w"], g["fc_w"])
    wcb = _pack_wcb(g["se_w1"], g["se_w2"], g["fm_w"], g["fm_b"],
                    g["bd_w"], g["bd_b"], g["cv_w"], g["cv_b"])
    cbl = _pack_cbl(g["fuse_b"], g["fc_b"])

    x8 = np.asarray(x, np.float32).astype(F8)
    y8 = np.asarray(y, np.float32).astype(F8)
    B = x8.shape[0]

    # x: 8-row overlapping windows, stride 6, partition r*16+ic
    xpad = np.zeros((B, 16, 6 * NT + 8, W), F8)
    xpad[:, :, 1:H + 1, :] = x8
    ridx = 6 * np.arange(NT)[:, None] + np.arange(8)[None, :]
    xp = xpad[:, :, ridx, :].transpose(0, 2, 3, 1, 4).reshape(B, NT, 128, W)

    # yh: [20, 2*NT, W]: partition r4*5+c, col-block 2t+k = ypad row 6t+4k+r4
    ypad = np.zeros((B, 5, 6 * NT + 8, W), F8)
    ypad[:, :, 1:H + 1, :] = y8
    yh = ypad[:, :, ridx, :]                     # [B, 5, NT, 8, W]
    yh = yh.reshape(B, 5, NT, 2, 4, W).transpose(0, 4, 1, 2, 3, 5) \
           .reshape(B, 20, 2 * NT, W)

    # yo: [NT, 32, W]: rows i*5+c = y row 6s+i; row 30 ones; row 31 zero
    yo = np.zeros((B, NT, 32, W), F8)
    cidx = 6 * np.arange(NT)[:, None] + 1 + np.arange(6)[None, :]
    yv = ypad[:, :, cidx, :]                     # [B, 5, NT, 6, W]
    yo[:, :, 0:30, :] = yv.transpose(0, 2, 3, 1, 4).reshape(B, NT, 30, W)
    yo[:, :, 30, :] = 1.0

    in_maps = [
        {"xp": np.ascontiguousarray(xp[i]),
         "yh": np.ascontiguousarray(yh[i]),
         "yo": np.ascontiguousarray(yo[i]),
         "wq8": wq8, "wcb": wcb, "cbl": cbl}
        for i in range(B)
    ]
    return nc, in_maps


def kernel(x, y, fuse_w, fuse_b, se_w1, se_w2, bd_w, bd_b,
           fc_w, fc_b, fm_w, fm_b, cv_w, cv_b):
    global LAST_RESULT
    from concourse.bass_utils import run_bass_kernel_spmd

    nc, in_maps = prepare(x, y, fuse_w, fuse_b, se_w1, se_w2, bd_w, bd_b,
                          fc_w, fc_b, fm_w, fm_b, cv_w, cv_b)
    res = run_bass_kernel_spmd(nc, in_maps, core_ids=list(range(8)))
    LAST_RESULT = res
    outs = []
    for i in range(8):
        ot = np.asarray(res.results[i]["out"], np.float32)  # [NO, 128, W]
        full = ot.reshape(NO, 8, 16, W).transpose(2, 0, 1, 3) \
                 .reshape(16, NO * 8, W)
        outs.append(full)
    return np.stack(outs)


# revision 34
# speedup vs baseline: 1.0637x; 1.0277x over previous
"""Trainium2 Bass kernel for nn_Boundary_Enchance (dense_cnn), v3.

Pure data parallel: core i of 8 processes batch image i.  Heavy compute runs
in fp8 (e4m3) DoubleRow matmuls at 0.5 PE-cycles/row; weights are scaled x8
on the host so they stay in e4m3's normal range, and the x8 is folded into
downstream coefficients or the sigmoid scale.

Per-core pipeline:
  phase 1 (interleaved, PAIR-batched): per 8-row tile t (stride 6 rows)
    fuse_box' = relu(8*(1x1conv(y)) + 8*b) as ONE fp8 DoubleRow matmul
    (k-tiles = rows 0-3 / 4-7, K=20 each); two tiles share a 2-bank PSUM
    tile and ONE evacuation op (relu+bias+accum row sums, Act/DVE
    alternating) writes both F-halves of the [x | F] slots of the XF
    mega-tile (fp8).  One pair behind, the 3x3 conv front for strips
    (2c, 2c+1) runs as 6 fp8 DoubleRow matmuls (3 dx taps per strip;
    k-tile0 = x window, k-tile1 = F window, SAME padding via
    partial-column psum accumulation) into a 2-bank PSUM tile, evacuated
    by ONE relu+bias op into the FC mega-tile (fp8).
    Edge handling: uniform bias everywhere; tile0-row0 fixed by a memset,
    tile85's phantom bias rows are subtracted from the row-sum column.
  SE chain: row sums -> selection matmul -> gap -> MLP -> sigmoid -> se;
    4 DVE scalar_tensor_tensor ops build group-position LBM lhsT blocks.
  phase 2 (one-group lag): per 4-strip group q: four M=128 accumulating
    LBM matmuls put x8 logits in function-major 32-partition blocks of one
    PSUM bank (u | v | v' | ones); ONE sigmoid (scale=1/8) -> sg; DVE
    copies u to a twin tile at base 64 and ONE DVE min computes
    min(u, v') for all 4 strips (min(u+v,1) = v + min(u, 1-v));
    three M=128 8-row LC matmuls expand scalefactor -> 16 channels
    (bias via the sigma(160/8)=1 ones row) into 2-bank PSUM pair tiles,
    ONE evacuation per out-tile pair -> bf16 og ring; DMA out per 4 tiles.
"""

import numpy as np
import ml_dtypes

F8 = ml_dtypes.float8_e4m3
BF16 = ml_dtypes.bfloat16

H = 512
W = 512
SB = 6                      # conv strip rows
NT = (H + SB - 1) // SB     # 86 strips / fuse tiles
NPAIR = NT // 2             # 43 tile pairs
NO = 64                     # 8-row output tiles
NG = (NT + 3) // 4          # 22 tail groups (last has 2 strips)
NPIX = float(H * W)

# fp8 const block column offsets (wq8)
LFJ_C = 0          # fuse lhsT [20, 2, 128] -> 256 cols
WXD_C = 256        # conv lhsT 3 x [128, 2, 96] -> 3 * 192
WQ8_W = 256 + 3 * 192

# bf16 const block column offsets (wcb)
SEL_C = 0
W1L_C = 16
W2R_C = 32
LC_C = 160         # 3 x 128
PSB_C = LC_C + 384  # 4 x 128
LM_C = PSB_C + 512  # 4 x 128
WCB_W = LM_C + 512

_cache = {}


# ----------------------------------------------------------------------------
# host-side weight layout builders
# ----------------------------------------------------------------------------

def _fuse_lhsT(fuse_w):
    """[20, 256]: k-tile k, col r*16+oc <- 8*fuse_w[oc, c] at row r4*5+c
    (r = 4k + r4)."""
    out = np.zeros((20, 2, 128), np.float32)
    w8 = 8.0 * fuse_w[:, :, 0, 0]                # [16, 5]
    for r in range(8):
        k, r4 = divmod(r, 4)
        out[r4 * 5:r4 * 5 + 5, k, r * 16:r * 16 + 16] = w8.T
    return out.reshape(20, 256)


def _conv_lhsT(fc_w):
    """3 x [128, 192]: dx tap d: k-tile 0 = 8*fc_w[:, :16] row-Toeplitz over
    x rows, k-tile 1 = fc_w[:, 16:] over F' rows (F' = 8F)."""
    out = np.zeros((3, 128, 2, 96), np.float32)
    for dx in range(3):
        for i in range(SB):
            for ky in range(3):
                r = i + ky
                out[dx, r * 16:r * 16 + 16, 0, i * 16:i * 16 + 16] = \
                    8.0 * fc_w[:, 0:16, ky, dx].T
                out[dx, r * 16:r * 16 + 16, 1, i * 16:i * 16 + 16] = \
                    fc_w[:, 16:32, ky, dx].T
    return out.reshape(3, 128, 192)


def _lbm_static(fm_w, fm_b, bd_w, bd_b):
    """PSB_g / LM_g [4][128, 128] (bf16 inputs to the on-device stt):
    logits carry an extra x8; the sigmoid un-scales with scale=1/8.
    col layout: 8g+i = u (mask), 32+8g+i = v (boundary), 64+8g+i = v',
    96 = ones row -> 160."""
    fm_d = fm_w[1, :, 0, 0] - fm_w[0, :, 0, 0]
    bd_d = 8.0 * (bd_w[1, :, 0, 0] - bd_w[0, :, 0, 0])
    fm_bd = 8.0 * (fm_b[1] - fm_b[0])
    bd_bd = 8.0 * (bd_b[1] - bd_b[0])
    psb = np.zeros((4, 128, 128), np.float32)
    lm = np.zeros((4, 128, 128), np.float32)
    for g in range(4):
        for i in range(SB):
            for oc in range(16):
                lm[g, i * 16 + oc, 8 * g + i] = fm_d[oc]
            lm[g, 126, 8 * g + i] = fm_bd
            for c in range(5):
                psb[g, 96 + i * 5 + c, 32 + 8 * g + i] = bd_d[c]
                psb[g, 96 + i * 5 + c, 64 + 8 * g + i] = -bd_d[c]
            lm[g, 126, 32 + 8 * g + i] = bd_bd
            lm[g, 126, 64 + 8 * g + i] = -bd_bd
    lm[0, 126, 96] = 160.0
    return psb, lm


def _lc_lhsT(cv_w, cv_b):
    """3 x [128, 128]: phase p covers group rows r = 8p..8p+7; strip-in-group
    g = r//6, i = r%6: sf = v + min(u, v')."""
    w = cv_w[:, 0, 0, 0]
    b = cv_b
    out = np.zeros((3, 128, 128), np.float32)
    for p in range(3):
        for r8 in range(8):
            r = 8 * p + r8
            g, i = divmod(r, SB)
            for oc in range(16):
                m = r8 * 16 + oc
                out[p, 32 + 8 * g + i, m] = w[oc]
                out[p, 64 + 8 * g + i, m] = w[oc]
                out[p, 96, m] = b[oc]
    return out


def _sel_lhsT():
    """[128, 16]: sum valid rows 1..6 of each tile / (8 * NPIX)."""
    out = np.zeros((128, 16), np.float32)
    for r in range(1, 7):
        for fc in range(16):
            out[r * 16 + fc, fc] = 1.0 / (8.0 * NPIX)
    return out


def _w1_lhsT(se_w1):
    out = np.zeros((128, 16), np.float32)
    out[:16, :16] = se_w1.T
    return out


def _w2_lhsT(se_w2):
    """[128, 128]: se logits at out partitions 96 + i*5 + c."""
    out = np.zeros((128, 128), np.float32)
    for i in range(SB):
        for c in range(5):
            out[:16, 96 + i * 5 + c] = se_w2[c]
    return out


def _pack_wq8(fuse_w, fc_w):
    out = np.zeros((128, WQ8_W), np.float32)
    out[0:20, LFJ_C:LFJ_C + 256] = _fuse_lhsT(fuse_w)
    wxd = _conv_lhsT(fc_w)
    for d in range(3):
        out[:, WXD_C + 192 * d:WXD_C + 192 * (d + 1)] = wxd[d]
    return out.astype(F8)


def _pack_wcb(se_w1, se_w2, fm_w, fm_b, bd_w, bd_b, cv_w, cv_b):
    out = np.zeros((128, WCB_W), np.float32)
    out[:, SEL_C:SEL_C + 16] = _sel_lhsT()
    out[:, W1L_C:W1L_C + 16] = _w1_lhsT(se_w1)
    out[:, W2R_C:W2R_C + 128] = _w2_lhsT(se_w2)
    lc = _lc_lhsT(cv_w, cv_b)
    for p in range(3):
        out[:, LC_C + 128 * p:LC_C + 128 * (p + 1)] = lc[p]
    psb, lm = _lbm_static(fm_w, fm_b, bd_w, bd_b)
    for g in range(4):
        out[:, PSB_C + 128 * g:PSB_C + 128 * (g + 1)] = psb[g]
        out[:, LM_C + 128 * g:LM_C + 128 * (g + 1)] = lm[g]
    return out.astype(BF16)


def _pack_cbl(fuse_b, fc_b):
    """[128, 4] f32 bias columns: 0 = 8*fuse_b (per row-group), 1 = 8*fc_b
    (conv), 2 = tile-85 phantom row-sum correction, 3 spare."""
    out = np.zeros((128, 4), np.float32)
    for r in range(8):
        out[r * 16:r * 16 + 16, 0] = 8.0 * fuse_b
    for i in range(SB):
        out[i * 16:i * 16 + 16, 1] = 8.0 * fc_b
    relu8b = np.maximum(8.0 * fuse_b, 0.0).astype(F8).astype(np.float32)
    for r in range(3, 7):
        out[r * 16:r * 16 + 16, 2] = float(W) * relu8b
    return out


# ----------------------------------------------------------------------------
# bass graph
# ----------------------------------------------------------------------------

def _build():
    import concourse.bass as bass
    import concourse.bacc as bacc
    import concourse.tile as tile
    from concourse import mybir

    f32 = mybir.dt.float32
    bf16 = mybir.dt.bfloat16
    f8 = mybir.dt.float8e4
    AF = mybir.ActivationFunctionType
    ALU = mybir.AluOpType
    DR = mybir.MatmulPerfMode.DoubleRow

    nc = bacc.Bacc("TRN2", target_bir_lowering=False)
    xp_ext = nc.declare_dram_parameter("xp", [NT, 128, W], f8, isOutput=False)
    yh_ext = nc.declare_dram_parameter("yh", [20, 2 * NT, W], f8,
                                       isOutput=False)
    yo_ext = nc.declare_dram_parameter("yo", [NT, 32, W], f8, isOutput=False)
    wq8_ext = nc.declare_dram_parameter("wq8", [128, WQ8_W], f8,
                                        isOutput=False)
    wcb_ext = nc.declare_dram_parameter("wcb", [128, WCB_W], bf16,
                                        isOutput=False)
    cbl_ext = nc.declare_dram_parameter("cbl", [128, 4], f32, isOutput=False)
    out_ext = nc.declare_dram_parameter("out", [NO, 128, W], bf16,
                                        isOutput=True)

    NYH = 32   # yh ring slots (1024 cols each)
    NSG = 4
    NOGP = 6   # og ring pair slots (1024 cols each)

    with tile.TileContext(nc) as tc:
        with (
            tc.tile_pool(name="singles", bufs=1) as singles,
            tc.tile_pool(name="pa", bufs=2, space="PSUM") as pa,
            tc.tile_pool(name="pb", bufs=2, space="PSUM") as pb,
        ):
            wq8 = singles.tile([128, WQ8_W], f8, tag="wq8")
            nc.sync.dma_start(out=wq8[:, :], in_=wq8_ext[:, :])
            wcb = singles.tile([128, WCB_W], bf16, tag="wcb")
            nc.sync.dma_start(out=wcb[:, :], in_=wcb_ext[:, :])
            cbl = singles.tile([128, 4], f32, tag="cbl")
            nc.sync.dma_start(out=cbl[:, :], in_=cbl_ext[:, :])

            LFJ = wq8[0:20, LFJ_C:LFJ_C + 256].rearrange(
                "p (two m) -> p two m", two=2)
            WXD = [wq8[:, WXD_C + 192 * d:WXD_C + 192 * (d + 1)].rearrange(
                "p (two m) -> p two m", two=2) for d in range(3)]
            SEL = wcb[:, SEL_C:SEL_C + 16]
            W1L = wcb[:, W1L_C:W1L_C + 16]
            W2R = wcb[:, W2R_C:W2R_C + 128]
            LC = [wcb[:, LC_C + 128 * p:LC_C + 128 * (p + 1)]
                  for p in range(3)]
            PSB = [wcb[:, PSB_C + 128 * g:PSB_C + 128 * (g + 1)]
                   for g in range(4)]
            LM = [wcb[:, LM_C + 128 * g:LM_C + 128 * (g + 1)]
                  for g in range(4)]

            XF = singles.tile([128, NT * 1024 + 512], f8, tag="XF", name="XF")
            FC = singles.tile([128, NT * W], f8, tag="FC", name="FC")
            YH = singles.tile([20, NYH * 1024], f8, tag="YH", name="YH")
            OG = singles.tile([128, NOGP * 1024], bf16, tag="OG", name="OG")
            SG = [singles.tile([128, 1024], bf16, tag=f"SG{k}", name=f"SG{k}")
                  for k in range(NSG)]
            SGU = [singles.tile([128, 1024], bf16, tag=f"SGU{k}", name=f"SGU{k}")
                   for k in range(NSG)]
            Ra = singles.tile([128, NPAIR], f32, tag="Ra")
            nc.vector.memset(Ra[:, :], 0.0)
            LBMG = [singles.tile([128, 256], f8, tag=f"LBM{g}",
                                 name=f"LBM{g}") for g in range(2)]

            # ================= phase 1: fuse + conv fronts ==================
            def issue_in_dma(j):
                t0 = 8 * j
                if t0 >= NT:
                    return
                n = min(8, NT - t0)
                s0 = t0 % NYH
                nc.gpsimd.dma_start(
                    out=YH[0:20, s0 * 1024:(s0 + n) * 1024].rearrange(
                        "p (s j) -> p s j", s=2 * n),
                    in_=yh_ext[:, 2 * t0:2 * (t0 + n), :])
                nc.gpsimd.dma_start(
                    out=XF[:, t0 * 1024:(t0 + n) * 1024].rearrange(
                        "p (s j) -> p s j", s=n)[:, :, 0:W],
                    in_=xp_ext[t0:t0 + n, :, :].rearrange("s p j -> p s j"))
                nc.gpsimd.dma_start(
                    out=FC[96:128, t0 * W:(t0 + n) * W].rearrange(
                        "p (s j) -> p s j", s=n),
                    in_=yo_ext[t0:t0 + n, :, :].rearrange("s p j -> p s j"))

            def issue_fuse_pair(k):
                t0 = 2 * k
                if t0 % 8 == 0:
                    issue_in_dma(t0 // 8 + 3)
                fps = pa.tile([128, 1024], f32, tag="a")
                for h in range(2):
                    t = t0 + h
                    s = t % NYH
                    rhs = YH[0:20, s * 1024:(s + 1) * 1024].rearrange(
                        "p (two j) -> p two j", two=2)
                    nc.tensor.matmul(fps[:, h * W:(h + 1) * W], lhsT=LFJ,
                                     rhs=rhs, start=True, stop=True,
                                     perf_mode=DR)
                # one evac for both halves -> F-halves of XF slots t0, t0+1
                dst = XF[:, t0 * 1024 + W:t0 * 1024 + W + 2048].rearrange(
                    "p (s j) -> p s j", s=2)[:, :, 0:W]
                nc.vector.tensor_scalar(out=dst, in0=fps[:, :],
                                        scalar1=cbl[:, 0:1],
                                        scalar2=0.0,
                                        op0=ALU.add, op1=ALU.max,
                                        accum_out=Ra[:, k:k + 1])
                if k == 0:
                    # tile0 row0 is image row -1: kill its bias-only relu
                    nc.vector.memset(XF[0:16, W:2 * W], 0.0)

            def issue_front_pair(c):
                cps = pb.tile([96, 1024], f32, tag="b")
                for h in range(2):
                    s = 2 * c + h
                    v = XF[:, s * 1024:(s + 1) * 1024].rearrange(
                        "p (two j) -> p two j", two=2)
                    o = h * W
                    nc.tensor.matmul(cps[:, o:o + W], lhsT=WXD[1],
                                     rhs=v[:, :, 0:W],
                                     start=True, stop=False, perf_mode=DR)
                    nc.tensor.matmul(cps[:, o + 1:o + W], lhsT=WXD[0],
                                     rhs=v[:, :, 0:W - 1],
                                     start=False, stop=False, perf_mode=DR)
                    nc.tensor.matmul(cps[:, o:o + W - 1], lhsT=WXD[2],
                                     rhs=v[:, :, 1:W],
                                     start=False, stop=True, perf_mode=DR)
                dst = FC[0:96, 2 * c * W:(2 * c + 2) * W]
                nc.scalar.activation(out=dst, in_=cps[:, :], func=AF.Relu,
                                     bias=cbl[0:96, 1:2])

            # ================= SE chain =====================================
            def issue_se():
                # tile-85 phantom bias rows leaked into Ra col 42: subtract
                nc.vector.tensor_scalar(out=Ra[:, NPAIR - 1:NPAIR],
                                        in0=Ra[:, NPAIR - 1:NPAIR],
                                        scalar1=cbl[:, 2:3], scalar2=0.0,
                                        op0=ALU.subtract, op1=ALU.add)
                Rbf = singles.tile([128, NPAIR], bf16, tag="Rbf")
                nc.vector.tensor_copy(out=Rbf[:, :], in_=Ra[:, :])
                gps = pb.tile([16, NPAIR], f32, tag="b")
                nc.tensor.matmul(gps[:, :], lhsT=SEL, rhs=Rbf[:, :],
                                 start=True, stop=True)
                gap_f = singles.tile([16, 1], f32, tag="gapf")
                nc.vector.reduce_sum(out=gap_f[:, :], in_=gps[:, :],
                                     axis=mybir.AxisListType.X)
                gap_bf = singles.tile([128, 1], bf16, tag="gap")
                nc.vector.memset(gap_bf[:, :], 0.0)
                nc.vector.tensor_copy(out=gap_bf[0:16, :], in_=gap_f[:, :])
                hps = pb.tile([16, 1], f32, tag="b")
                nc.tensor.matmul(hps[:, :], lhsT=W1L, rhs=gap_bf[:, :],
                                 start=True, stop=True)
                h_bf = singles.tile([128, 1], bf16, tag="hbf")
                nc.vector.memset(h_bf[:, :], 0.0)
                nc.scalar.activation(out=h_bf[0:16, :], in_=hps[:, :],
                                     func=AF.Relu)
                sps = pb.tile([128, 1], f32, tag="b")
                nc.tensor.matmul(sps[:, :], lhsT=W2R, rhs=h_bf[:, :],
                                 start=True, stop=True)
                se_bc = singles.tile([128, 1], f32, tag="sebc")
                nc.scalar.activation(out=se_bc[:, :], in_=sps[:, :],
                                     func=AF.Sigmoid)
                for g in range(2):
                    nc.vector.scalar_tensor_tensor(
                        out=LBMG[g][:, :],
                        in0=wcb[:, PSB_C + 256 * g:PSB_C + 256 * (g + 1)],
                        scalar=se_bc[:, :],
                        in1=wcb[:, LM_C + 256 * g:LM_C + 256 * (g + 1)],
                        op0=ALU.mult, op1=ALU.add)

            # ================= phase 2: tails ===============================
            cvt = {}
            p2ctr = [0]

            def p2tile(name):
                i = p2ctr[0]
                p2ctr[0] += 1
                pool = pa if i % 2 == 0 else pb
                return pool.tile([128, 1024], f32,
                                 tag=("a" if i % 2 == 0 else "b"), name=name)

            def issue_head_pair(j):
                mb = p2tile(f"mb{j}")
                for h in range(2):
                    q = 2 * j + h
                    ns = min(4, NT - 4 * q)
                    nm = ns // 2
                    for g2 in range(nm):
                        u0 = 4 * q + 2 * g2
                        nc.tensor.matmul(
                            mb[:, h * W:(h + 1) * W],
                            lhsT=LBMG[g2][:, :].rearrange(
                                "p (two m) -> p two m", two=2),
                            rhs=FC[:, u0 * W:(u0 + 2) * W].rearrange(
                                "p (two j) -> p two j", two=2),
                            start=(g2 == 0), stop=(g2 == nm - 1),
                            perf_mode=DR)
                sg = SG[j % NSG]
                sgu = SGU[j % NSG]
                nc.scalar.activation(out=sg[:, :], in_=mb[:, :],
                                     func=AF.Sigmoid, scale=0.125)
                nc.vector.tensor_copy(out=sgu[64:96, :], in_=sg[0:32, :])
                nc.vector.tensor_tensor(out=sg[64:96, :], in0=sgu[64:96, :],
                                        in1=sg[64:96, :], op=ALU.min)

            def issue_head_pair_split(j):
                mb = pb.tile([128, 1024], f32, tag="b", name=f"mb{j}")
                sg = SG[j % NSG]
                sgu = SGU[j % NSG]
                for h in range(2):
                    q = 2 * j + h
                    ns = min(4, NT - 4 * q)
                    for g in range(ns):
                        u = 4 * q + g
                        nc.tensor.matmul(mb[:, h * W:(h + 1) * W],
                                         lhsT=LBMG[g][:, :],
                                         rhs=FC[:, u * W:(u + 1) * W],
                                         start=(g == 0), stop=(g == ns - 1))
                    hv = slice(h * W, (h + 1) * W)
                    nc.scalar.activation(out=sg[:, hv], in_=mb[:, hv],
                                         func=AF.Sigmoid, scale=0.125)
                nc.vector.tensor_copy(out=sgu[64:96, :], in_=sg[0:32, :])
                nc.vector.tensor_tensor(out=sg[64:96, :], in0=sgu[64:96, :],
                                        in1=sg[64:96, :], op=ALU.min)

            def issue_lc(q):
                sg = SG[(q // 2) % NSG][:, (q % 2) * W:(q % 2 + 1) * W]
                np_ = 3 if q < NG - 1 else 1
                for p in range(np_):
                    tau = 3 * q + p
                    pi, h = divmod(tau, 2)
                    if h == 0:
                        cvt[pi] = p2tile(f"cv{pi}")
                    ops = cvt[pi]
                    nc.tensor.matmul(ops[:, h * W:(h + 1) * W], lhsT=LC[p],
                                     rhs=sg, start=True, stop=True)
                    if h == 1:
                        dst = OG[:, (pi % NOGP) * 1024:
                                 (pi % NOGP + 1) * 1024]
                        if pi % 2 == 0:
                            nc.scalar.activation(out=dst, in_=ops[:, :],
                                                 func=AF.Copy)
                        else:
                            nc.vector.tensor_copy(out=dst, in_=ops[:, :])
                        del cvt[pi]
                        if pi % 3 == 2 or pi == 31:
                            p0 = pi - (pi % 3)
                            t0 = 2 * p0
                            n = 2 * (pi - p0 + 1)
                            c0 = (p0 % NOGP) * 1024
                            nc.sync.dma_start(
                                out=out_ext[t0:t0 + n, :, :].rearrange(
                                    "s p j -> p s j"),
                                in_=OG[:, c0:c0 + n * W].rearrange(
                                    "p (s j) -> p s j", s=n))

            for j in range(3):
                issue_in_dma(j)
            for k in range(NPAIR + 2):
                if k < NPAIR:
                    issue_fuse_pair(k)
                if k >= 2:
                    issue_front_pair(k - 2)
            issue_se()
            NJ = (NG + 1) // 2
            for j in range(NJ + 2):
                if j < NJ:
                    issue_head_pair(j)
                if j >= 2:
                    issue_lc(2 * (j - 2))
                    issue_lc(2 * (j - 2) + 1)
    nc.compile()
    return nc


# ----------------------------------------------------------------------------
# entry point
# ----------------------------------------------------------------------------

LAST_RESULT = None


def prepare(x, y, fuse_w, fuse_b, se_w1, se_w2, bd_w, bd_b,
            fc_w, fc_b, fm_w, fm_b, cv_w, cv_b):
    if "nc" not in _cache:
        _cache["nc"] = _build()
    nc = _cache["nc"]

    g = {}
    for k, v in (("fuse_w", fuse_w), ("fuse_b", fuse_b), ("se_w1", se_w1),
                 ("se_w2", se_w2), ("bd_w", bd_w), ("bd_b", bd_b),
                 ("fc_w", fc_w), ("fc_b", fc_b), ("fm_w", fm_w),
                 ("fm_b", fm_b), ("cv_w", cv_w), ("cv_b", cv_b)):
        g[k] = np.asarray(v, np.float32)

    wq8 = _pack_wq8(g["fuse_w"], g["fc_w"])
    wcb = _pack_wcb(g["se_w1"], g["se_w2"], g["fm_w"], g["fm_b"],
                    g["bd_w"], g["bd_b"], g["cv_w"], g["cv_b"])
    cbl = _pack_cbl(g["fuse_b"], g["fc_b"])

    x8 = np.asarray(x, np.float32).astype(F8)
    y8 = np.asarray(y, np.float32).astype(F8)
    B = x8.shape[0]

    # x: 8-row overlapping windows, stride 6, partition r*16+ic
    xpad = np.zeros((B, 16, 6 * NT + 8, W), F8)
    xpad[:, :, 1:H + 1, :] = x8
    ridx = 6 * np.arange(NT)[:, None] + np.arange(8)[None, :]
    xp = xpad[:, :, ridx, :].transpose(0, 2, 3, 1, 4).reshape(B, NT, 128, W)

    # yh: [20, 2*NT, W]: partition r4*5+c, col-block 2t+k = ypad row 6t+4k+r4
    ypad = np.zeros((B, 5, 6 * NT + 8, W), F8)
    ypad[:, :, 1:H + 1, :] = y8
    yh = ypad[:, :, ridx, :]                     # [B, 5, NT, 8, W]
    yh = yh.reshape(B, 5, NT, 2, 4, W).transpose(0, 4, 1, 2, 3, 5) \
           .reshape(B, 20, 2 * NT, W)

    # yo: [NT, 32, W]: rows i*5+c = y row 6s+i; row 30 ones; row 31 zero
    yo = np.zeros((B, NT, 32, W), F8)
    cidx = 6 * np.arange(NT)[:, None] + 1 + np.arange(6)[None, :]
    yv = ypad[:, :, cidx, :]                     # [B, 5, NT, 6, W]
    yo[:, :, 0:30, :] = yv.transpose(0, 2, 3, 1, 4).reshape(B, NT, 30, W)
    yo[:, :, 30, :] = 1.0

    in_maps = [
        {"xp": np.ascontiguousarray(xp[i]),
         "yh": np.ascontiguousarray(yh[i]),
         "yo": np.ascontiguousarray(yo[i]),
         "wq8": wq8, "wcb": wcb, "cbl": cbl}
        for i in range(B)
    ]
    return nc, in_maps


def kernel(x, y, fuse_w, fuse_b, se_w1, se_w2, bd_w, bd_b,
           fc_w, fc_b, fm_w, fm_b, cv_w, cv_b):
    global LAST_RESULT
    from concourse.bass_utils import run_bass_kernel_spmd

    nc, in_maps = prepare(x, y, fuse_w, fuse_b, se_w1, se_w2, bd_w, bd_b,
                          fc_w, fc_b, fm_w, fm_b, cv_w, cv_b)
    res = run_bass_kernel_spmd(nc, in_maps, core_ids=list(range(8)))
    LAST_RESULT = res
    outs = []
    for i in range(8):
        ot = np.asarray(res.results[i]["out"], np.float32)  # [NO, 128, W]
        full = ot.reshape(NO, 8, 16, W).transpose(2, 0, 1, 3) \
                 .reshape(16, NO * 8, W)
        outs.append(full)
    return np.stack(outs)


# revision 43
# speedup vs baseline: 1.1313x; 1.0635x over previous
"""Trainium2 Bass kernel for nn_Boundary_Enchance (dense_cnn), v3.

Pure data parallel: core i of 8 processes batch image i.  Heavy compute runs
in fp8 (e4m3) DoubleRow matmuls at 0.5 PE-cycles/row; weights are scaled x8
on the host so they stay in e4m3's normal range, and the x8 is folded into
downstream coefficients or the sigmoid scale.

Per-core pipeline:
  phase 1 (interleaved, PAIR-batched): per 8-row tile t (stride 6 rows)
    fuse_box' = relu(8*(1x1conv(y)) + 8*b) as ONE fp8 DoubleRow matmul
    (k-tiles = rows 0-3 / 4-7, K=20 each); two tiles share a 2-bank PSUM
    tile and ONE evacuation op (relu+bias+accum row sums, Act/DVE
    alternating) writes both F-halves of the [x | F] slots of the XF
    mega-tile (fp8).  One pair behind, the 3x3 conv front for strips
    (2c, 2c+1) runs as 6 fp8 DoubleRow matmuls (3 dx taps per strip;
    k-tile0 = x window, k-tile1 = F window, SAME padding via
    partial-column psum accumulation) into a 2-bank PSUM tile, evacuated
    by ONE relu+bias op into the FC mega-tile (fp8).
    Edge handling: uniform bias everywhere; tile0-row0 fixed by a memset,
    tile85's phantom bias rows are subtracted from the row-sum column.
  SE chain: row sums -> selection matmul -> gap -> MLP -> sigmoid -> se;
    4 DVE scalar_tensor_tensor ops build group-position LBM lhsT blocks.
  phase 2 (one-group lag): per 4-strip group q: four M=128 accumulating
    LBM matmuls put x8 logits in function-major 32-partition blocks of one
    PSUM bank (u | v | v' | ones); ONE sigmoid (scale=1/8) -> sg; DVE
    copies u to a twin tile at base 64 and ONE DVE min computes
    min(u, v') for all 4 strips (min(u+v,1) = v + min(u, 1-v));
    three M=128 8-row LC matmuls expand scalefactor -> 16 channels
    (bias via the sigma(160/8)=1 ones row) into 2-bank PSUM pair tiles,
    ONE evacuation per out-tile pair -> bf16 og ring; DMA out per 4 tiles.
"""

import numpy as np
import ml_dtypes

F8 = ml_dtypes.float8_e4m3
BF16 = ml_dtypes.bfloat16

H = 512
W = 512
SB = 6                      # conv strip rows
NT = (H + SB - 1) // SB     # 86 strips / fuse tiles
NPAIR = NT // 2             # 43 tile pairs
NO = 64                     # 8-row output tiles
NG = (NT + 3) // 4          # 22 tail groups (last has 2 strips)
NPIX = float(H * W)

# fp8 const block column offsets (wq8)
LFJ_C = 0          # fuse lhsT [20, 2, 128] -> 256 cols
WXD_C = 256        # conv lhsT 3 x [128, 2, 96] -> 3 * 192
WQ8_W = 256 + 3 * 192

# bf16 const block column offsets (wcb)
SEL_C = 0
W1L_C = 16
W2R_C = 32
LC_C = 160         # 3 x 128
PSB_C = LC_C + 384  # 4 x 128
LM_C = PSB_C + 512  # 4 x 128
WCB_W = LM_C + 512

_cache = {}


# ----------------------------------------------------------------------------
# host-side weight layout builders
# ----------------------------------------------------------------------------

def _fuse_lhsT(fuse_w):
    """[20, 256]: k-tile k, col r*16+oc <- 8*fuse_w[oc, c] at row r4*5+c
    (r = 4k + r4)."""
    out = np.zeros((20, 2, 128), np.float32)
    w8 = 8.0 * fuse_w[:, :, 0, 0]                # [16, 5]
    for r in range(8):
        k, r4 = divmod(r, 4)
        out[r4 * 5:r4 * 5 + 5, k, r * 16:r * 16 + 16] = w8.T
    return out.reshape(20, 256)


def _conv_lhsT(fc_w):
    """3 x [128, 192]: dx tap d: k-tile 0 = 8*fc_w[:, :16] row-Toeplitz over
    x rows, k-tile 1 = fc_w[:, 16:] over F' rows (F' = 8F)."""
    out = np.zeros((3, 128, 2, 96), np.float32)
    for dx in range(3):
        for i in range(SB):
            for ky in range(3):
                r = i + ky
                out[dx, r * 16:r * 16 + 16, 0, i * 16:i * 16 + 16] = \
                    8.0 * fc_w[:, 0:16, ky, dx].T
                out[dx, r * 16:r * 16 + 16, 1, i * 16:i * 16 + 16] = \
                    fc_w[:, 16:32, ky, dx].T
    return out.reshape(3, 128, 192)


def _lbm_static(fm_w, fm_b, bd_w, bd_b):
    """PSB_g / LM_g [4][128, 128] (bf16 inputs to the on-device stt):
    logits carry an extra x8; the sigmoid un-scales with scale=1/8.
    col layout: 8g+i = u (mask), 32+8g+i = v (boundary), 64+8g+i = v',
    96 = ones row -> 160."""
    fm_d = fm_w[1, :, 0, 0] - fm_w[0, :, 0, 0]
    bd_d = 8.0 * (bd_w[1, :, 0, 0] - bd_w[0, :, 0, 0])
    fm_bd = 8.0 * (fm_b[1] - fm_b[0])
    bd_bd = 8.0 * (bd_b[1] - bd_b[0])
    psb = np.zeros((4, 128, 128), np.float32)
    lm = np.zeros((4, 128, 128), np.float32)
    for g in range(4):
        for i in range(SB):
            for oc in range(16):
                lm[g, i * 16 + oc, 8 * g + i] = fm_d[oc]
            lm[g, 126, 8 * g + i] = fm_bd
            for c in range(5):
                psb[g, 96 + i * 5 + c, 32 + 8 * g + i] = bd_d[c]
                psb[g, 96 + i * 5 + c, 64 + 8 * g + i] = -bd_d[c]
            lm[g, 126, 32 + 8 * g + i] = bd_bd
            lm[g, 126, 64 + 8 * g + i] = -bd_bd
    lm[0, 126, 96] = 160.0
    return psb, lm


def _lc_lhsT(cv_w, cv_b):
    """3 x [128, 128]: phase p covers group rows r = 8p..8p+7; strip-in-group
    g = r//6, i = r%6: sf = v + min(u, v')."""
    w = cv_w[:, 0, 0, 0]
    b = cv_b
    out = np.zeros((3, 128, 128), np.float32)
    for p in range(3):
        for r8 in range(8):
            r = 8 * p + r8
            g, i = divmod(r, SB)
            for oc in range(16):
                m = r8 * 16 + oc
                out[p, 32 + 8 * g + i, m] = w[oc]
                out[p, 64 + 8 * g + i, m] = w[oc]
                out[p, 96, m] = b[oc]
    return out


def _sel_lhsT():
    """[128, 16]: sum valid rows 1..6 of each tile / (8 * NPIX)."""
    out = np.zeros((128, 16), np.float32)
    for r in range(1, 7):
        for fc in range(16):
            out[r * 16 + fc, fc] = 1.0 / (8.0 * NPIX)
    return out


def _w1_lhsT(se_w1):
    out = np.zeros((128, 16), np.float32)
    out[:16, :16] = se_w1.T
    return out


def _w2_lhsT(se_w2):
    """[128, 128]: se logits at out partitions 96 + i*5 + c."""
    out = np.zeros((128, 128), np.float32)
    for i in range(SB):
        for c in range(5):
            out[:16, 96 + i * 5 + c] = se_w2[c]
    return out


def _pack_wq8(fuse_w, fc_w):
    out = np.zeros((128, WQ8_W), np.float32)
    out[0:20, LFJ_C:LFJ_C + 256] = _fuse_lhsT(fuse_w)
    wxd = _conv_lhsT(fc_w)
    for d in range(3):
        out[:, WXD_C + 192 * d:WXD_C + 192 * (d + 1)] = wxd[d]
    return out.astype(F8)


def _pack_wcb(se_w1, se_w2, fm_w, fm_b, bd_w, bd_b, cv_w, cv_b):
    out = np.zeros((128, WCB_W), np.float32)
    out[:, SEL_C:SEL_C + 16] = _sel_lhsT()
    out[:, W1L_C:W1L_C + 16] = _w1_lhsT(se_w1)
    out[:, W2R_C:W2R_C + 128] = _w2_lhsT(se_w2)
    lc = _lc_lhsT(cv_w, cv_b)
    for p in range(3):
        out[:, LC_C + 128 * p:LC_C + 128 * (p + 1)] = lc[p]
    psb, lm = _lbm_static(fm_w, fm_b, bd_w, bd_b)
    for g in range(4):
        out[:, PSB_C + 128 * g:PSB_C + 128 * (g + 1)] = psb[g]
        out[:, LM_C + 128 * g:LM_C + 128 * (g + 1)] = lm[g]
    return out.astype(BF16)


def _pack_cbl(fuse_b, fc_b):
    """[128, 4] f32 bias columns: 0 = 8*fuse_b (per row-group), 1 = 8*fc_b
    (conv), 2 = tile-85 phantom row-sum correction, 3 spare."""
    out = np.zeros((128, 4), np.float32)
    for r in range(8):
        out[r * 16:r * 16 + 16, 0] = 8.0 * fuse_b
    for i in range(SB):
        out[i * 16:i * 16 + 16, 1] = 8.0 * fc_b
    relu8b = np.maximum(8.0 * fuse_b, 0.0).astype(F8).astype(np.float32)
    for r in range(3, 7):
        out[r * 16:r * 16 + 16, 2] = float(W) * relu8b
    return out


# ----------------------------------------------------------------------------
# bass graph
# ----------------------------------------------------------------------------

def _build():
    import concourse.bass as bass
    import concourse.bacc as bacc
    import concourse.tile as tile
    from concourse import mybir

    f32 = mybir.dt.float32
    bf16 = mybir.dt.bfloat16
    f8 = mybir.dt.float8e4
    AF = mybir.ActivationFunctionType
    ALU = mybir.AluOpType
    DR = mybir.MatmulPerfMode.DoubleRow

    nc = bacc.Bacc("TRN2", target_bir_lowering=False)
    xp_ext = nc.declare_dram_parameter("xp", [NT, 128, W], f8, isOutput=False)
    yh_ext = nc.declare_dram_parameter("yh", [20, 2 * NT, W], f8,
                                       isOutput=False)
    yo_ext = nc.declare_dram_parameter("yo", [NT, 32, W], f8, isOutput=False)
    wq8_ext = nc.declare_dram_parameter("wq8", [128, WQ8_W], f8,
                                        isOutput=False)
    wcb_ext = nc.declare_dram_parameter("wcb", [128, WCB_W], bf16,
                                        isOutput=False)
    cbl_ext = nc.declare_dram_parameter("cbl", [128, 4], f32, isOutput=False)
    out_ext = nc.declare_dram_parameter("out", [NO, 128, W], bf16,
                                        isOutput=True)

    NYH = 32   # yh ring slots (1024 cols each)
    NSG = 4
    NOGP = 9   # og ring pair slots (1024 cols each)

    with tile.TileContext(nc) as tc:
        with (
            tc.tile_pool(name="singles", bufs=1) as singles,
            tc.tile_pool(name="pa", bufs=2, space="PSUM") as pa,
            tc.tile_pool(name="pb", bufs=2, space="PSUM") as pb,
        ):
            wq8 = singles.tile([128, WQ8_W], f8, tag="wq8")
            nc.sync.dma_start(out=wq8[:, :], in_=wq8_ext[:, :])
            wcb = singles.tile([128, WCB_W], bf16, tag="wcb")
            nc.sync.dma_start(out=wcb[:, :], in_=wcb_ext[:, :])
            cbl = singles.tile([128, 4], f32, tag="cbl")
            nc.sync.dma_start(out=cbl[:, :], in_=cbl_ext[:, :])

            LFJ = wq8[0:20, LFJ_C:LFJ_C + 256].rearrange(
                "p (two m) -> p two m", two=2)
            WXD = [wq8[:, WXD_C + 192 * d:WXD_C + 192 * (d + 1)].rearrange(
                "p (two m) -> p two m", two=2) for d in range(3)]
            SEL = wcb[:, SEL_C:SEL_C + 16]
            W1L = wcb[:, W1L_C:W1L_C + 16]
            W2R = wcb[:, W2R_C:W2R_C + 128]
            LC = [wcb[:, LC_C + 128 * p:LC_C + 128 * (p + 1)]
                  for p in range(3)]
            PSB = [wcb[:, PSB_C + 128 * g:PSB_C + 128 * (g + 1)]
                   for g in range(4)]
            LM = [wcb[:, LM_C + 128 * g:LM_C + 128 * (g + 1)]
                  for g in range(4)]

            XF = singles.tile([128, NT * 1024 + 512], f8, tag="XF", name="XF")
            FC = singles.tile([128, NT * W], f8, tag="FC", name="FC")
            YH = singles.tile([20, NYH * 1024], f8, tag="YH", name="YH")
            OG = singles.tile([128, NOGP * 1024], bf16, tag="OG", name="OG")
            SG = [singles.tile([128, 1024], bf16, tag=f"SG{k}", name=f"SG{k}")
                  for k in range(NSG)]
            SGU = [singles.tile([128, 1024], bf16, tag=f"SGU{k}", name=f"SGU{k}")
                   for k in range(NSG)]
            Ra = singles.tile([128, NPAIR], f32, tag="Ra")
            nc.vector.memset(Ra[:, :], 0.0)
            LBMG = [singles.tile([128, 256], f8, tag=f"LBM{g}",
                                 name=f"LBM{g}") for g in range(2)]

            # ================= phase 1: fuse + conv fronts ==================
            def issue_in_dma(j):
                t0 = 8 * j
                if t0 >= NT:
                    return
                n = min(8, NT - t0)
                s0 = t0 % NYH
                nc.gpsimd.dma_start(
                    out=YH[0:20, s0 * 1024:(s0 + n) * 1024].rearrange(
                        "p (s j) -> p s j", s=2 * n),
                    in_=yh_ext[:, 2 * t0:2 * (t0 + n), :])
                nc.gpsimd.dma_start(
                    out=XF[:, t0 * 1024:(t0 + n) * 1024].rearrange(
                        "p (s j) -> p s j", s=n)[:, :, 0:W],
                    in_=xp_ext[t0:t0 + n, :, :].rearrange("s p j -> p s j"))
                nc.gpsimd.dma_start(
                    out=FC[96:128, t0 * W:(t0 + n) * W].rearrange(
                        "p (s j) -> p s j", s=n),
                    in_=yo_ext[t0:t0 + n, :, :].rearrange("s p j -> p s j"))

            def issue_fuse_pair(k):
                t0 = 2 * k
                if t0 % 8 == 0:
                    issue_in_dma(t0 // 8 + 3)
                fps = pa.tile([128, 1024], f32, tag="a")
                for h in range(2):
                    t = t0 + h
                    s = t % NYH
                    rhs = YH[0:20, s * 1024:(s + 1) * 1024].rearrange(
                        "p (two j) -> p two j", two=2)
                    nc.tensor.matmul(fps[:, h * W:(h + 1) * W], lhsT=LFJ,
                                     rhs=rhs, start=True, stop=True,
                                     perf_mode=DR)
                # one evac for both halves -> F-halves of XF slots t0, t0+1
                dst = XF[:, t0 * 1024 + W:t0 * 1024 + W + 2048].rearrange(
                    "p (s j) -> p s j", s=2)[:, :, 0:W]
                nc.vector.tensor_scalar(out=dst, in0=fps[:, :],
                                        scalar1=cbl[:, 0:1],
                                        scalar2=0.0,
                                        op0=ALU.add, op1=ALU.max,
                                        accum_out=Ra[:, k:k + 1])
                if k == 0:
                    # tile0 row0 is image row -1: kill its bias-only relu
                    nc.vector.memset(XF[0:16, W:2 * W], 0.0)

            def issue_front_pair(c):
                cps = pb.tile([96, 1024], f32, tag="b")
                for h in range(2):
                    s = 2 * c + h
                    v = XF[:, s * 1024:(s + 1) * 1024].rearrange(
                        "p (two j) -> p two j", two=2)
                    o = h * W
                    nc.tensor.matmul(cps[:, o:o + W], lhsT=WXD[1],
                                     rhs=v[:, :, 0:W],
                                     start=True, stop=False, perf_mode=DR)
                    nc.tensor.matmul(cps[:, o + 1:o + W], lhsT=WXD[0],
                                     rhs=v[:, :, 0:W - 1],
                                     start=False, stop=False, perf_mode=DR)
                    nc.tensor.matmul(cps[:, o:o + W - 1], lhsT=WXD[2],
                                     rhs=v[:, :, 1:W],
                                     start=False, stop=True, perf_mode=DR)
                dst = FC[0:96, 2 * c * W:(2 * c + 2) * W]
                nc.scalar.activation(out=dst, in_=cps[:, :], func=AF.Relu,
                                     bias=cbl[0:96, 1:2])

            # ================= SE chain =====================================
            def issue_se():
                # tile-85 phantom bias rows leaked into Ra col 42: subtract
                nc.vector.tensor_scalar(out=Ra[:, NPAIR - 1:NPAIR],
                                        in0=Ra[:, NPAIR - 1:NPAIR],
                                        scalar1=cbl[:, 2:3], scalar2=0.0,
                                        op0=ALU.subtract, op1=ALU.add)
                Rbf = singles.tile([128, NPAIR], bf16, tag="Rbf")
                nc.vector.tensor_copy(out=Rbf[:, :], in_=Ra[:, :])
                gps = pb.tile([16, NPAIR], f32, tag="b")
                nc.tensor.matmul(gps[:, :], lhsT=SEL, rhs=Rbf[:, :],
                                 start=True, stop=True)
                gap_f = singles.tile([16, 1], f32, tag="gapf")
                nc.vector.reduce_sum(out=gap_f[:, :], in_=gps[:, :],
                                     axis=mybir.AxisListType.X)
                gap_bf = singles.tile([128, 1], bf16, tag="gap")
                nc.vector.memset(gap_bf[:, :], 0.0)
                nc.vector.tensor_copy(out=gap_bf[0:16, :], in_=gap_f[:, :])
                hps = pb.tile([16, 1], f32, tag="b")
                nc.tensor.matmul(hps[:, :], lhsT=W1L, rhs=gap_bf[:, :],
                                 start=True, stop=True)
                h_bf = singles.tile([128, 1], bf16, tag="hbf")
                nc.vector.memset(h_bf[:, :], 0.0)
                nc.scalar.activation(out=h_bf[0:16, :], in_=hps[:, :],
                                     func=AF.Relu)
                sps = pb.tile([128, 1], f32, tag="b")
                nc.tensor.matmul(sps[:, :], lhsT=W2R, rhs=h_bf[:, :],
                                 start=True, stop=True)
                se_bc = singles.tile([128, 1], f32, tag="sebc")
                nc.scalar.activation(out=se_bc[:, :], in_=sps[:, :],
                                     func=AF.Sigmoid)
                for g in range(2):
                    nc.vector.scalar_tensor_tensor(
                        out=LBMG[g][:, :],
                        in0=wcb[:, PSB_C + 256 * g:PSB_C + 256 * (g + 1)],
                        scalar=se_bc[:, :],
                        in1=wcb[:, LM_C + 256 * g:LM_C + 256 * (g + 1)],
                        op0=ALU.mult, op1=ALU.add)

            # ================= phase 2: tails ===============================
            cvt = {}
            p2ctr = [0]

            def p2tile(name):
                i = p2ctr[0]
                p2ctr[0] += 1
                pool = pa if i % 2 == 0 else pb
                return pool.tile([128, 1024], f32,
                                 tag=("a" if i % 2 == 0 else "b"), name=name)

            def issue_head_pair(j):
                mb = p2tile(f"mb{j}")
                for h in range(2):
                    q = 2 * j + h
                    ns = min(4, NT - 4 * q)
                    nm = ns // 2
                    for g2 in range(nm):
                        u0 = 4 * q + 2 * g2
                        nc.tensor.matmul(
                            mb[:, h * W:(h + 1) * W],
                            lhsT=LBMG[g2][:, :].rearrange(
                                "p (two m) -> p two m", two=2),
                            rhs=FC[:, u0 * W:(u0 + 2) * W].rearrange(
                                "p (two j) -> p two j", two=2),
                            start=(g2 == 0), stop=(g2 == nm - 1),
                            perf_mode=DR)
                sg = SG[j % NSG]
                sgu = SGU[j % NSG]
                nc.scalar.activation(out=sg[:, :], in_=mb[:, :],
                                     func=AF.Sigmoid, scale=0.125)
                nc.vector.tensor_copy(out=sgu[64:96, :], in_=sg[0:32, :])
                nc.vector.tensor_tensor(out=sg[64:96, :], in0=sgu[64:96, :],
                                        in1=sg[64:96, :], op=ALU.min)

            def issue_head_pair_split(j):
                mb = pb.tile([128, 1024], f32, tag="b", name=f"mb{j}")
                sg = SG[j % NSG]
                sgu = SGU[j % NSG]
                for h in range(2):
                    q = 2 * j + h
                    ns = min(4, NT - 4 * q)
                    for g in range(ns):
                        u = 4 * q + g
                        nc.tensor.matmul(mb[:, h * W:(h + 1) * W],
                                         lhsT=LBMG[g][:, :],
                                         rhs=FC[:, u * W:(u + 1) * W],
                                         start=(g == 0), stop=(g == ns - 1))
                    hv = slice(h * W, (h + 1) * W)
                    nc.scalar.activation(out=sg[:, hv], in_=mb[:, hv],
                                         func=AF.Sigmoid, scale=0.125)
                nc.vector.tensor_copy(out=sgu[64:96, :], in_=sg[0:32, :])
                nc.vector.tensor_tensor(out=sg[64:96, :], in0=sgu[64:96, :],
                                        in1=sg[64:96, :], op=ALU.min)

            def issue_lc(q):
                sg = SG[(q // 2) % NSG][:, (q % 2) * W:(q % 2 + 1) * W]
                np_ = 3 if q < NG - 1 else 1
                for p in range(np_):
                    tau = 3 * q + p
                    pi, h = divmod(tau, 2)
                    if h == 0:
                        cvt[pi] = p2tile(f"cv{pi}")
                    ops = cvt[pi]
                    nc.tensor.matmul(ops[:, h * W:(h + 1) * W], lhsT=LC[p],
                                     rhs=sg, start=True, stop=True)
                    if h == 1:
                        dst = OG[:, (pi % NOGP) * 1024:
                                 (pi % NOGP + 1) * 1024]
                        if pi % 2 == 0:
                            nc.scalar.activation(out=dst, in_=ops[:, :],
                                                 func=AF.Copy)
                        else:
                            nc.vector.tensor_copy(out=dst, in_=ops[:, :])
                        del cvt[pi]
                        if pi % 3 == 2 or pi == 31:
                            p0 = pi - (pi % 3)
                            t0 = 2 * p0
                            n = 2 * (pi - p0 + 1)
                            c0 = (p0 % NOGP) * 1024
                            nc.sync.dma_start(
                                out=out_ext[t0:t0 + n, :, :].rearrange(
                                    "s p j -> p s j"),
                                in_=OG[:, c0:c0 + n * W].rearrange(
                                    "p (s j) -> p s j", s=n))

            for j in range(3):
                issue_in_dma(j)
            for k in range(NPAIR + 2):
                if k < NPAIR:
                    issue_fuse_pair(k)
                if k >= 2:
                    issue_front_pair(k - 2)
            issue_se()
            NJ = (NG + 1) // 2
            for j in range(NJ + 2):
                if j < NJ:
                    issue_head_pair(j)
                if j >= 2:
                    issue_lc(2 * (j - 2))
                    issue_lc(2 * (j - 2) + 1)
    nc.compile()
    return nc


# ----------------------------------------------------------------------------
# entry point
# ----------------------------------------------------------------------------

LAST_RESULT = None


def prepare(x, y, fuse_w, fuse_b, se_w1, se_w2, bd_w, bd_b,
            fc_w, fc_b, fm_w, fm_b, cv_w, cv_b):
    if "nc" not in _cache:
        _cache["nc"] = _build()
    nc = _cache["nc"]

    g = {}
    for k, v in (("fuse_w", fuse_w), ("fuse_b", fuse_b), ("se_w1", se_w1),
                 ("se_w2", se_w2), ("bd_w", bd_w), ("bd_b", bd_b),
                 ("fc_w", fc_w), ("fc_b", fc_b), ("fm_w", fm_w),
                 ("fm_b", fm_b), ("cv_w", cv_w), ("cv_b", cv_b)):
        g[k] = np.asarray(v, np.float32)

    wq8 = _pack_wq8(g["fuse_w"], g["fc_w"])
    wcb = _pack_wcb(g["se_w1"], g["se_w2"], g["fm_w"], g["fm_b"],
                    g["bd_w"], g["bd_b"], g["cv_w"], g["cv_b"])
    cbl = _pack_cbl(g["fuse_b"], g["fc_b"])

    x8 = np.asarray(x, np.float32).astype(F8)
    y8 = np.asarray(y, np.float32).astype(F8)
    B = x8.shape[0]

    # x: 8-row overlapping windows, stride 6, partition r*16+ic
    xpad = np.zeros((B, 16, 6 * NT + 8, W), F8)
    xpad[:, :, 1:H + 1, :] = x8
    ridx = 6 * np.arange(NT)[:, None] + np.arange(8)[None, :]
    xp = xpad[:, :, ridx, :].transpose(0, 2, 3, 1, 4).reshape(B, NT, 128, W)

    # yh: [20, 2*NT, W]: partition r4*5+c, col-block 2t+k = ypad row 6t+4k+r4
    ypad = np.zeros((B, 5, 6 * NT + 8, W), F8)
    ypad[:, :, 1:H + 1, :] = y8
    yh = ypad[:, :, ridx, :]                     # [B, 5, NT, 8, W]
    yh = yh.reshape(B, 5, NT, 2, 4, W).transpose(0, 4, 1, 2, 3, 5) \
           .reshape(B, 20, 2 * NT, W)

    # yo: [NT, 32, W]: rows i*5+c = y row 6s+i; row 30 ones; row 31 zero
    yo = np.zeros((B, NT, 32, W), F8)
    cidx = 6 * np.arange(NT)[:, None] + 1 + np.arange(6)[None, :]
    yv = ypad[:, :, cidx, :]                     # [B, 5, NT, 6, W]
    yo[:, :, 0:30, :] = yv.transpose(0, 2, 3, 1, 4).reshape(B, NT, 30, W)
    yo[:, :, 30, :] = 1.0

    in_maps = [
        {"xp": np.ascontiguousarray(xp[i]),
         "yh": np.ascontiguousarray(yh[i]),
         "yo": np.ascontiguousarray(yo[i]),
         "wq8": wq8, "wcb": wcb, "cbl": cbl}
        for i in range(B)
    ]
    return nc, in_maps


def kernel(x, y, fuse_w, fuse_b, se_w1, se_w2, bd_w, bd_b,
           fc_w, fc_b, fm_w, fm_b, cv_w, cv_b):
    global LAST_RESULT
    from concourse.bass_utils import run_bass_kernel_spmd

    nc, in_maps = prepare(x, y, fuse_w, fuse_b, se_w1, se_w2, bd_w, bd_b,
                          fc_w, fc_b, fm_w, fm_b, cv_w, cv_b)
    res = run_bass_kernel_spmd(nc, in_maps, core_ids=list(range(8)))
    LAST_RESULT = res
    outs = []
    for i in range(8):
        ot = np.asarray(res.results[i]["out"], np.float32)  # [NO, 128, W]
        full = ot.reshape(NO, 8, 16, W).transpose(2, 0, 1, 3) \
                 .reshape(16, NO * 8, W)
        outs.append(full)
    return np.stack(outs)


# revision 49
# speedup vs baseline: 1.1485x; 1.0152x over previous
"""Trainium2 Bass kernel for nn_Boundary_Enchance (dense_cnn), v3.

Pure data parallel: core i of 8 processes batch image i.  Heavy compute runs
in fp8 (e4m3) DoubleRow matmuls at 0.5 PE-cycles/row; weights are scaled x8
on the host so they stay in e4m3's normal range, and the x8 is folded into
downstream coefficients or the sigmoid scale.

Per-core pipeline:
  phase 1 (interleaved, PAIR-batched): per 8-row tile t (stride 6 rows)
    fuse_box' = relu(8*(1x1conv(y)) + 8*b) as ONE fp8 DoubleRow matmul
    (k-tiles = rows 0-3 / 4-7, K=20 each); two tiles share a 2-bank PSUM
    tile and ONE evacuation op (relu+bias+accum row sums, Act/DVE
    alternating) writes both F-halves of the [x | F] slots of the XF
    mega-tile (fp8).  One pair behind, the 3x3 conv front for strips
    (2c, 2c+1) runs as 6 fp8 DoubleRow matmuls (3 dx taps per strip;
    k-tile0 = x window, k-tile1 = F window, SAME padding via
    partial-column psum accumulation) into a 2-bank PSUM tile, evacuated
    by ONE relu+bias op into the FC mega-tile (fp8).
    Edge handling: uniform bias everywhere; tile0-row0 fixed by a memset,
    tile85's phantom bias rows are subtracted from the row-sum column.
  SE chain: row sums -> selection matmul -> gap -> MLP -> sigmoid -> se;
    4 DVE scalar_tensor_tensor ops build group-position LBM lhsT blocks.
  phase 2 (one-group lag): per 4-strip group q: four M=128 accumulating
    LBM matmuls put x8 logits in function-major 32-partition blocks of one
    PSUM bank (u | v | v' | ones); ONE sigmoid (scale=1/8) -> sg; DVE
    copies u to a twin tile at base 64 and ONE DVE min computes
    min(u, v') for all 4 strips (min(u+v,1) = v + min(u, 1-v));
    three M=128 8-row LC matmuls expand scalefactor -> 16 channels
    (bias via the sigma(160/8)=1 ones row) into 2-bank PSUM pair tiles,
    ONE evacuation per out-tile pair -> bf16 og ring; DMA out per 4 tiles.
"""

import numpy as np
import ml_dtypes

F8 = ml_dtypes.float8_e4m3
BF16 = ml_dtypes.bfloat16

H = 512
W = 512
SB = 6                      # conv strip rows
NT = (H + SB - 1) // SB     # 86 strips / fuse tiles
NPAIR = NT // 2             # 43 tile pairs
NO = 64                     # 8-row output tiles
NG = (NT + 3) // 4          # 22 tail groups (last has 2 strips)
NPIX = float(H * W)

# fp8 const block column offsets (wq8)
LFJ_C = 0          # fuse lhsT [20, 2, 128] -> 256 cols
WXD_C = 256        # conv lhsT 3 x [128, 2, 96] -> 3 * 192
WQ8_W = 256 + 3 * 192

# bf16 const block column offsets (wcb)
SEL_C = 0
W1L_C = 16
W2R_C = 32
LC_C = 160         # 3 x 128
PSB_C = LC_C + 384  # 4 x 128
LM_C = PSB_C + 512  # 4 x 128
WCB_W = LM_C + 512

_cache = {}


# ----------------------------------------------------------------------------
# host-side weight layout builders
# ----------------------------------------------------------------------------

def _fuse_lhsT(fuse_w):
    """[20, 256]: k-tile k, col r*16+oc <- 8*fuse_w[oc, c] at row r4*5+c
    (r = 4k + r4)."""
    out = np.zeros((20, 2, 128), np.float32)
    w8 = 8.0 * fuse_w[:, :, 0, 0]                # [16, 5]
    for r in range(8):
        k, r4 = divmod(r, 4)
        out[r4 * 5:r4 * 5 + 5, k, r * 16:r * 16 + 16] = w8.T
    return out.reshape(20, 256)


def _conv_lhsT(fc_w):
    """3 x [128, 192]: dx tap d: k-tile 0 = 8*fc_w[:, :16] row-Toeplitz over
    x rows, k-tile 1 = fc_w[:, 16:] over F' rows (F' = 8F)."""
    out = np.zeros((3, 128, 2, 96), np.float32)
    for dx in range(3):
        for i in range(SB):
            for ky in range(3):
                r = i + ky
                out[dx, r * 16:r * 16 + 16, 0, i * 16:i * 16 + 16] = \
                    8.0 * fc_w[:, 0:16, ky, dx].T
                out[dx, r * 16:r * 16 + 16, 1, i * 16:i * 16 + 16] = \
                    fc_w[:, 16:32, ky, dx].T
    return out.reshape(3, 128, 192)


def _lbm_static(fm_w, fm_b, bd_w, bd_b):
    """PSB_g / LM_g [4][128, 128] (bf16 inputs to the on-device stt):
    logits carry an extra x8; the sigmoid un-scales with scale=1/8.
    col layout: 8g+i = u (mask), 32+8g+i = v (boundary), 64+8g+i = v',
    96 = ones row -> 160."""
    fm_d = fm_w[1, :, 0, 0] - fm_w[0, :, 0, 0]
    bd_d = 8.0 * (bd_w[1, :, 0, 0] - bd_w[0, :, 0, 0])
    fm_bd = 8.0 * (fm_b[1] - fm_b[0])
    bd_bd = 8.0 * (bd_b[1] - bd_b[0])
    psb = np.zeros((4, 128, 128), np.float32)
    lm = np.zeros((4, 128, 128), np.float32)
    for g in range(4):
        for i in range(SB):
            for oc in range(16):
                lm[g, i * 16 + oc, 8 * g + i] = fm_d[oc]
            lm[g, 126, 8 * g + i] = fm_bd
            for c in range(5):
                psb[g, 96 + i * 5 + c, 32 + 8 * g + i] = bd_d[c]
                psb[g, 96 + i * 5 + c, 64 + 8 * g + i] = -bd_d[c]
            lm[g, 126, 32 + 8 * g + i] = bd_bd
            lm[g, 126, 64 + 8 * g + i] = -bd_bd
    lm[0, 126, 96] = 160.0
    return psb, lm


def _lc_lhsT(cv_w, cv_b):
    """3 x [128, 128]: phase p covers group rows r = 8p..8p+7; strip-in-group
    g = r//6, i = r%6: sf = v + min(u, v')."""
    w = cv_w[:, 0, 0, 0]
    b = cv_b
    out = np.zeros((3, 128, 128), np.float32)
    for p in range(3):
        for r8 in range(8):
            r = 8 * p + r8
            g, i = divmod(r, SB)
            for oc in range(16):
                m = r8 * 16 + oc
                out[p, 32 + 8 * g + i, m] = w[oc]
                out[p, 64 + 8 * g + i, m] = w[oc]
                out[p, 96, m] = b[oc]
    return out


def _sel_lhsT():
    """[128, 16]: sum valid rows 1..6 of each tile / (8 * NPIX)."""
    out = np.zeros((128, 16), np.float32)
    for r in range(1, 7):
        for fc in range(16):
            out[r * 16 + fc, fc] = 1.0 / (8.0 * NPIX)
    return out


def _w1_lhsT(se_w1):
    out = np.zeros((128, 16), np.float32)
    out[:16, :16] = se_w1.T
    return out


def _w2_lhsT(se_w2):
    """[128, 128]: se logits at out partitions 96 + i*5 + c."""
    out = np.zeros((128, 128), np.float32)
    for i in range(SB):
        for c in range(5):
            out[:16, 96 + i * 5 + c] = se_w2[c]
    return out


def _pack_wq8(fuse_w, fc_w):
    out = np.zeros((128, WQ8_W), np.float32)
    out[0:20, LFJ_C:LFJ_C + 256] = _fuse_lhsT(fuse_w)
    wxd = _conv_lhsT(fc_w)
    for d in range(3):
        out[:, WXD_C + 192 * d:WXD_C + 192 * (d + 1)] = wxd[d]
    return out.astype(F8)


def _pack_wcb(se_w1, se_w2, fm_w, fm_b, bd_w, bd_b, cv_w, cv_b):
    out = np.zeros((128, WCB_W), np.float32)
    out[:, SEL_C:SEL_C + 16] = _sel_lhsT()
    out[:, W1L_C:W1L_C + 16] = _w1_lhsT(se_w1)
    out[:, W2R_C:W2R_C + 128] = _w2_lhsT(se_w2)
    lc = _lc_lhsT(cv_w, cv_b)
    for p in range(3):
        out[:, LC_C + 128 * p:LC_C + 128 * (p + 1)] = lc[p]
    psb, lm = _lbm_static(fm_w, fm_b, bd_w, bd_b)
    for g in range(4):
        out[:, PSB_C + 128 * g:PSB_C + 128 * (g + 1)] = psb[g]
        out[:, LM_C + 128 * g:LM_C + 128 * (g + 1)] = lm[g]
    return out.astype(BF16)


def _pack_cbl(fuse_b, fc_b):
    """[128, 4] f32 bias columns: 0 = 8*fuse_b (per row-group), 1 = 8*fc_b
    (conv), 2 = tile-85 phantom row-sum correction, 3 spare."""
    out = np.zeros((128, 4), np.float32)
    for r in range(8):
        out[r * 16:r * 16 + 16, 0] = 8.0 * fuse_b
    for i in range(SB):
        out[i * 16:i * 16 + 16, 1] = 8.0 * fc_b
    relu8b = np.maximum(8.0 * fuse_b, 0.0).astype(F8).astype(np.float32)
    for r in range(3, 7):
        out[r * 16:r * 16 + 16, 2] = float(W) * relu8b
    return out


# ----------------------------------------------------------------------------
# bass graph
# ----------------------------------------------------------------------------

def _build():
    import concourse.bass as bass
    import concourse.bacc as bacc
    import concourse.tile as tile
    from concourse import mybir

    f32 = mybir.dt.float32
    bf16 = mybir.dt.bfloat16
    f8 = mybir.dt.float8e4
    AF = mybir.ActivationFunctionType
    ALU = mybir.AluOpType
    DR = mybir.MatmulPerfMode.DoubleRow

    nc = bacc.Bacc("TRN2", target_bir_lowering=False)
    xp_ext = nc.declare_dram_parameter("xp", [NT, 128, W], f8, isOutput=False)
    yh_ext = nc.declare_dram_parameter("yh", [20, 2 * NT, W], f8,
                                       isOutput=False)
    yo_ext = nc.declare_dram_parameter("yo", [NT, 32, W], f8, isOutput=False)
    wq8_ext = nc.declare_dram_parameter("wq8", [128, WQ8_W], f8,
                                        isOutput=False)
    wcb_ext = nc.declare_dram_parameter("wcb", [128, WCB_W], bf16,
                                        isOutput=False)
    cbl_ext = nc.declare_dram_parameter("cbl", [128, 4], f32, isOutput=False)
    out_ext = nc.declare_dram_parameter("out", [NO, 128, W], bf16,
                                        isOutput=True)

    NYH = 32   # yh ring slots (1024 cols each)
    NSG = 4
    NOGP = 9   # og ring pair slots (1024 cols each)

    with tile.TileContext(nc) as tc:
        with (
            tc.tile_pool(name="singles", bufs=1) as singles,
            tc.tile_pool(name="pa", bufs=2, space="PSUM") as pa,
            tc.tile_pool(name="pb", bufs=2, space="PSUM") as pb,
        ):
            wq8 = singles.tile([128, WQ8_W], f8, tag="wq8")
            nc.sync.dma_start(out=wq8[:, :], in_=wq8_ext[:, :])
            wcb = singles.tile([128, WCB_W], bf16, tag="wcb")
            nc.sync.dma_start(out=wcb[:, :], in_=wcb_ext[:, :])
            cbl = singles.tile([128, 4], f32, tag="cbl")
            nc.sync.dma_start(out=cbl[:, :], in_=cbl_ext[:, :])

            LFJ = wq8[0:20, LFJ_C:LFJ_C + 256].rearrange(
                "p (two m) -> p two m", two=2)
            WXD = [wq8[:, WXD_C + 192 * d:WXD_C + 192 * (d + 1)].rearrange(
                "p (two m) -> p two m", two=2) for d in range(3)]
            SEL = wcb[:, SEL_C:SEL_C + 16]
            W1L = wcb[:, W1L_C:W1L_C + 16]
            W2R = wcb[:, W2R_C:W2R_C + 128]
            LC = [wcb[:, LC_C + 128 * p:LC_C + 128 * (p + 1)]
                  for p in range(3)]
            PSB = [wcb[:, PSB_C + 128 * g:PSB_C + 128 * (g + 1)]
                   for g in range(4)]
            LM = [wcb[:, LM_C + 128 * g:LM_C + 128 * (g + 1)]
                  for g in range(4)]

            XF = singles.tile([128, NT * 1024 + 512], f8, tag="XF", name="XF")
            FC = singles.tile([128, NT * W], f8, tag="FC", name="FC")
            YH = singles.tile([20, NYH * 1024], f8, tag="YH", name="YH")
            OG = singles.tile([128, NOGP * 1024], bf16, tag="OG", name="OG")
            SG = [singles.tile([128, 1024], bf16, tag=f"SG{k}", name=f"SG{k}")
                  for k in range(NSG)]
            SGU = [singles.tile([128, 1024], bf16, tag=f"SGU{k}", name=f"SGU{k}")
                   for k in range(NSG)]
            Ra = singles.tile([128, NPAIR], f32, tag="Ra")
            nc.vector.memset(Ra[:, :], 0.0)
            LBMG = [singles.tile([128, 256], f8, tag=f"LBM{g}",
                                 name=f"LBM{g}") for g in range(2)]

            # ================= phase 1: fuse + conv fronts ==================
            def issue_in_dma(j):
                t0 = 8 * j
                if t0 >= NT:
                    return
                n = min(8, NT - t0)
                s0 = t0 % NYH
                nc.gpsimd.dma_start(
                    out=YH[0:20, s0 * 1024:(s0 + n) * 1024].rearrange(
                        "p (s j) -> p s j", s=2 * n),
                    in_=yh_ext[:, 2 * t0:2 * (t0 + n), :])
                nc.gpsimd.dma_start(
                    out=XF[:, t0 * 1024:(t0 + n) * 1024].rearrange(
                        "p (s j) -> p s j", s=n)[:, :, 0:W],
                    in_=xp_ext[t0:t0 + n, :, :].rearrange("s p j -> p s j"))
                nc.gpsimd.dma_start(
                    out=FC[96:128, t0 * W:(t0 + n) * W].rearrange(
                        "p (s j) -> p s j", s=n),
                    in_=yo_ext[t0:t0 + n, :, :].rearrange("s p j -> p s j"))

            def issue_fuse_pair(k):
                t0 = 2 * k
                if t0 % 8 == 0:
                    issue_in_dma(t0 // 8 + 3)
                fps = pa.tile([128, 1024], f32, tag="a")
                for h in range(2):
                    t = t0 + h
                    s = t % NYH
                    rhs = YH[0:20, s * 1024:(s + 1) * 1024].rearrange(
                        "p (two j) -> p two j", two=2)
                    nc.tensor.matmul(fps[:, h * W:(h + 1) * W], lhsT=LFJ,
                                     rhs=rhs, start=True, stop=True,
                                     perf_mode=DR)
                # one evac for both halves -> F-halves of XF slots t0, t0+1
                dst = XF[:, t0 * 1024 + W:t0 * 1024 + W + 2048].rearrange(
                    "p (s j) -> p s j", s=2)[:, :, 0:W]
                nc.vector.tensor_scalar(out=dst, in0=fps[:, :],
                                        scalar1=cbl[:, 0:1],
                                        scalar2=0.0,
                                        op0=ALU.add, op1=ALU.max,
                                        accum_out=Ra[:, k:k + 1])
                if k == 0:
                    # tile0 row0 is image row -1: kill its bias-only relu
                    nc.vector.memset(XF[0:16, W:2 * W], 0.0)

            def issue_front_pair(c):
                cps = pb.tile([96, 1024], f32, tag="b")
                for h in range(2):
                    s = 2 * c + h
                    v = XF[:, s * 1024:(s + 1) * 1024].rearrange(
                        "p (two j) -> p two j", two=2)
                    o = h * W
                    nc.tensor.matmul(cps[:, o:o + W], lhsT=WXD[1],
                                     rhs=v[:, :, 0:W],
                                     start=True, stop=False, perf_mode=DR)
                    nc.tensor.matmul(cps[:, o + 1:o + W], lhsT=WXD[0],
                                     rhs=v[:, :, 0:W - 1],
                                     start=False, stop=False, perf_mode=DR)
                    nc.tensor.matmul(cps[:, o:o + W - 1], lhsT=WXD[2],
                                     rhs=v[:, :, 1:W],
                                     start=False, stop=True, perf_mode=DR)
                dst = FC[0:96, 2 * c * W:(2 * c + 2) * W]
                nc.scalar.activation(out=dst, in_=cps[:, :], func=AF.Relu,
                                     bias=cbl[0:96, 1:2])

            # ================= SE chain =====================================
            def issue_se():
                # tile-85 phantom bias rows leaked into Ra col 42: subtract
                nc.vector.tensor_scalar(out=Ra[:, NPAIR - 1:NPAIR],
                                        in0=Ra[:, NPAIR - 1:NPAIR],
                                        scalar1=cbl[:, 2:3], scalar2=0.0,
                                        op0=ALU.subtract, op1=ALU.add)
                Rbf = singles.tile([128, NPAIR], bf16, tag="Rbf")
                nc.vector.tensor_copy(out=Rbf[:, :], in_=Ra[:, :])
                gps = pb.tile([16, NPAIR], f32, tag="b")
                nc.tensor.matmul(gps[:, :], lhsT=SEL, rhs=Rbf[:, :],
                                 start=True, stop=True)
                gap_f = singles.tile([16, 1], f32, tag="gapf")
                nc.vector.reduce_sum(out=gap_f[:, :], in_=gps[:, :],
                                     axis=mybir.AxisListType.X)
                gap_bf = singles.tile([128, 1], bf16, tag="gap")
                nc.vector.memset(gap_bf[:, :], 0.0)
                nc.vector.tensor_copy(out=gap_bf[0:16, :], in_=gap_f[:, :])
                hps = pb.tile([16, 1], f32, tag="b")
                nc.tensor.matmul(hps[:, :], lhsT=W1L, rhs=gap_bf[:, :],
                                 start=True, stop=True)
                h_bf = singles.tile([128, 1], bf16, tag="hbf")
                nc.vector.memset(h_bf[:, :], 0.0)
                nc.scalar.activation(out=h_bf[0:16, :], in_=hps[:, :],
                                     func=AF.Relu)
                sps = pb.tile([128, 1], f32, tag="b")
                nc.tensor.matmul(sps[:, :], lhsT=W2R, rhs=h_bf[:, :],
                                 start=True, stop=True)
                se_bc = singles.tile([128, 1], f32, tag="sebc")
                nc.scalar.activation(out=se_bc[:, :], in_=sps[:, :],
                                     func=AF.Sigmoid)
                for g in range(2):
                    nc.vector.scalar_tensor_tensor(
                        out=LBMG[g][:, :],
                        in0=wcb[:, PSB_C + 256 * g:PSB_C + 256 * (g + 1)],
                        scalar=se_bc[:, :],
                        in1=wcb[:, LM_C + 256 * g:LM_C + 256 * (g + 1)],
                        op0=ALU.mult, op1=ALU.add)

            # ================= phase 2: tails ===============================
            cvt = {}
            p2ctr = [0]

            def p2tile(name):
                i = p2ctr[0]
                p2ctr[0] += 1
                pool = pa if i % 2 == 0 else pb
                return pool.tile([128, 1024], f32,
                                 tag=("a" if i % 2 == 0 else "b"), name=name)

            def issue_head_pair(j):
                mb = p2tile(f"mb{j}")
                for h in range(2):
                    q = 2 * j + h
                    ns = min(4, NT - 4 * q)
                    nm = ns // 2
                    for g2 in range(nm):
                        u0 = 4 * q + 2 * g2
                        nc.tensor.matmul(
                            mb[:, h * W:(h + 1) * W],
                            lhsT=LBMG[g2][:, :].rearrange(
                                "p (two m) -> p two m", two=2),
                            rhs=FC[:, u0 * W:(u0 + 2) * W].rearrange(
                                "p (two j) -> p two j", two=2),
                            start=(g2 == 0), stop=(g2 == nm - 1),
                            perf_mode=DR)
                sg = SG[j % NSG]
                sgu = SGU[j % NSG]
                nc.scalar.activation(out=sg[:, :], in_=mb[:, :],
                                     func=AF.Sigmoid, scale=0.125)
                nc.vector.tensor_copy(out=sgu[64:96, :], in_=sg[0:32, :])
                nc.vector.tensor_tensor(out=sg[64:96, :], in0=sgu[64:96, :],
                                        in1=sg[64:96, :], op=ALU.min)

            def issue_head_pair_split(j):
                mb = pb.tile([128, 1024], f32, tag="b", name=f"mb{j}")
                sg = SG[j % NSG]
                sgu = SGU[j % NSG]
                for h in range(2):
                    q = 2 * j + h
                    ns = min(4, NT - 4 * q)
                    for g in range(ns):
                        u = 4 * q + g
                        nc.tensor.matmul(mb[:, h * W:(h + 1) * W],
                                         lhsT=LBMG[g][:, :],
                                         rhs=FC[:, u * W:(u + 1) * W],
                                         start=(g == 0), stop=(g == ns - 1))
                    hv = slice(h * W, (h + 1) * W)
                    nc.scalar.activation(out=sg[:, hv], in_=mb[:, hv],
                                         func=AF.Sigmoid, scale=0.125)
                nc.vector.tensor_copy(out=sgu[64:96, :], in_=sg[0:32, :])
                nc.vector.tensor_tensor(out=sg[64:96, :], in0=sgu[64:96, :],
                                        in1=sg[64:96, :], op=ALU.min)

            def issue_lc(q):
                sg = SG[(q // 2) % NSG][:, (q % 2) * W:(q % 2 + 1) * W]
                np_ = 3 if q < NG - 1 else 1
                for p in range(np_):
                    tau = 3 * q + p
                    pi, h = divmod(tau, 2)
                    if h == 0:
                        cvt[pi] = p2tile(f"cv{pi}")
                    ops = cvt[pi]
                    nc.tensor.matmul(ops[:, h * W:(h + 1) * W], lhsT=LC[p],
                                     rhs=sg, start=True, stop=True)
                    if h == 1:
                        dst = OG[:, (pi % NOGP) * 1024:
                                 (pi % NOGP + 1) * 1024]
                        if pi % 2 == 0:
                            nc.scalar.activation(out=dst, in_=ops[:, :],
                                                 func=AF.Copy)
                        else:
                            nc.vector.tensor_copy(out=dst, in_=ops[:, :])
                        del cvt[pi]
                        nc.sync.dma_start(
                            out=out_ext[2 * pi:2 * pi + 2, :, :].rearrange(
                                "s p j -> p s j"),
                            in_=OG[:, (pi % NOGP) * 1024:
                                   (pi % NOGP) * 1024 + 2 * W].rearrange(
                                "p (s j) -> p s j", s=2))

            for j in range(3):
                issue_in_dma(j)
            for k in range(NPAIR + 2):
                if k < NPAIR:
                    issue_fuse_pair(k)
                if k >= 2:
                    issue_front_pair(k - 2)
            issue_se()
            NJ = (NG + 1) // 2
            for j in range(NJ + 2):
                if j < NJ:
                    issue_head_pair(j)
                if j >= 2:
                    issue_lc(2 * (j - 2))
                    issue_lc(2 * (j - 2) + 1)
    nc.compile()
    return nc


# ----------------------------------------------------------------------------
# entry point
# ----------------------------------------------------------------------------

LAST_RESULT = None


def prepare(x, y, fuse_w, fuse_b, se_w1, se_w2, bd_w, bd_b,
            fc_w, fc_b, fm_w, fm_b, cv_w, cv_b):
    if "nc" not in _cache:
        _cache["nc"] = _build()
    nc = _cache["nc"]

    g = {}
    for k, v in (("fuse_w", fuse_w), ("fuse_b", fuse_b), ("se_w1", se_w1),
                 ("se_w2", se_w2), ("bd_w", bd_w), ("bd_b", bd_b),
                 ("fc_w", fc_w), ("fc_b", fc_b), ("fm_w", fm_w),
                 ("fm_b", fm_b), ("cv_w", cv_w), ("cv_b", cv_b)):
        g[k] = np.asarray(v, np.float32)

    wq8 = _pack_wq8(g["fuse_w"], g["fc_w"])
    wcb = _pack_wcb(g["se_w1"], g["se_w2"], g["fm_w"], g["fm_b"],
                    g["bd_w"], g["bd_b"], g["cv_w"], g["cv_b"])
    cbl = _pack_cbl(g["fuse_b"], g["fc_b"])

    x8 = np.asarray(x, np.float32).astype(F8)
    y8 = np.asarray(y, np.float32).astype(F8)
    B = x8.shape[0]

    # x: 8-row overlapping windows, stride 6, partition r*16+ic
    xpad = np.zeros((B, 16, 6 * NT + 8, W), F8)
    xpad[:, :, 1:H + 1, :] = x8
    ridx = 6 * np.arange(NT)[:, None] + np.arange(8)[None, :]
    xp = xpad[:, :, ridx, :].transpose(0, 2, 3, 1, 4).reshape(B, NT, 128, W)

    # yh: [20, 2*NT, W]: partition r4*5+c, col-block 2t+k = ypad row 6t+4k+r4
    ypad = np.zeros((B, 5, 6 * NT + 8, W), F8)
    ypad[:, :, 1:H + 1, :] = y8
    yh = ypad[:, :, ridx, :]                     # [B, 5, NT, 8, W]
    yh = yh.reshape(B, 5, NT, 2, 4, W).transpose(0, 4, 1, 2, 3, 5) \
           .reshape(B, 20, 2 * NT, W)

    # yo: [NT, 32, W]: rows i*5+c = y row 6s+i; row 30 ones; row 31 zero
    yo = np.zeros((B, NT, 32, W), F8)
    cidx = 6 * np.arange(NT)[:, None] + 1 + np.arange(6)[None, :]
    yv = ypad[:, :, cidx, :]                     # [B, 5, NT, 6, W]
    yo[:, :, 0:30, :] = yv.transpose(0, 2, 3, 1, 4).reshape(B, NT, 30, W)
    yo[:, :, 30, :] = 1.0

    in_maps = [
        {"xp": np.ascontiguousarray(xp[i]),
         "yh": np.ascontiguousarray(yh[i]),
         "yo": np.ascontiguousarray(yo[i]),
         "wq8": wq8, "wcb": wcb, "cbl": cbl}
        for i in range(B)
    ]
    return nc, in_maps


def kernel(x, y, fuse_w, fuse_b, se_w1, se_w2, bd_w, bd_b,
           fc_w, fc_b, fm_w, fm_b, cv_w, cv_b):
    global LAST_RESULT
    from concourse.bass_utils import run_bass_kernel_spmd

    nc, in_maps = prepare(x, y, fuse_w, fuse_b, se_w1, se_w2, bd_w, bd_b,
                          fc_w, fc_b, fm_w, fm_b, cv_w, cv_b)
    res = run_bass_kernel_spmd(nc, in_maps, core_ids=list(range(8)))
    LAST_RESULT = res
    outs = []
    for i in range(8):
        ot = np.asarray(res.results[i]["out"], np.float32)  # [NO, 128, W]
        full = ot.reshape(NO, 8, 16, W).transpose(2, 0, 1, 3) \
                 .reshape(16, NO * 8, W)
        outs.append(full)
    return np.stack(outs)
